# revision 1
# baseline (speedup 1.0000x reference)
"""Trainium2 Bass kernel for the Dblock-ViT channel-attention module.

Strategy: data-parallel over batch (8 batches -> 8 NeuronCores). Each core:
  q       = l2norm(text_emb[b] @ Wq.T + bq)              [C, HW]
  per branch i in 0..3:
    kk    = fused dense 3x3 conv (1x1 folded into taps)  [C, HW]
    vv    = same for the v path                          [C, HW]
    attn  = instancenorm(q_n @ kk_n.T / sqrt(C)); p = softmax rows
    out_i = (Wpo_i @ p) @ vv                             [C, HW]

Matmuls run fp32r (full-rate fp32 at N>=256); the attention contraction
(transposed operands) runs bf16. Conv inputs stream per 8-row slab with a
1-row halo so DMA double-buffers against the PE.
"""

import math
import sys
import types

import ml_dtypes
import numpy as np

BF16_NP = ml_dtypes.bfloat16

for _p in ("/opt/trn_rl_repo",):
    if _p not in sys.path:
        sys.path.insert(0, _p)

# The image's antenv package lacks axon_hooks; register a functional stand-in
# so run_bass_kernel_spmd(trace=True) can reach the NTFF profiling hook.
try:
    import antenv
    if "antenv.axon_hooks" not in sys.modules:
        _m = types.ModuleType("antenv.axon_hooks")
        _m._hook_val = None
        _m.set_axon_ntff_profile_hook = lambda h: setattr(_m, "_hook_val", h)
        _m.get_axon_ntff_profile_hook = lambda: _m._hook_val
        sys.modules["antenv.axon_hooks"] = _m
        antenv.axon_hooks = _m
        try:
            from trn_agent_boot.trn_boot import _ntff_profile_via_ctypes
            _m._hook_val = _ntff_profile_via_ctypes("/opt/axon/libaxon_pjrt.so")
        except Exception:
            pass
except Exception:
    pass

import concourse.bass as bass
import concourse.mybir as mybir
import concourse.tile as tile
from concourse import bacc, bass_utils
from concourse.masks import make_identity

try:
    bass_utils.upload_artifacts = lambda tmpdir: tmpdir
except Exception:
    pass

B, C, H, W = 8, 128, 128, 128
HW = H * W
TS = 512
KP = 5                      # 640 = 512 (text) + 1 (bias row) + padding
EPS_NORM = 1e-12
EPS_IN = 1e-5
RSQRT_C = 1.0 / math.sqrt(C)
F32 = mybir.dt.float32
F32R = mybir.dt.float32r
BF16 = mybir.dt.bfloat16
TAPS = [(dy, dx) for dy in range(3) for dx in range(3)]
AX = mybir.AxisListType
ALU = mybir.AluOpType
ACTF = mybir.ActivationFunctionType


def _body(nc, tc, textT_d, wqT_d, embs_d, weffk_d, weffv_d, wpoT_d, outs_d):
    from contextlib import ExitStack
    ctx = ExitStack()
    with ctx:
        singles = ctx.enter_context(tc.tile_pool(name="singles", bufs=1))
        small = ctx.enter_context(tc.tile_pool(name="small", bufs=1))
        med = ctx.enter_context(tc.tile_pool(name="med", bufs=2))
        scratch = ctx.enter_context(tc.tile_pool(name="scratch", bufs=2))
        stgp = ctx.enter_context(tc.tile_pool(name="stgp", bufs=3))
        outp = ctx.enter_context(tc.tile_pool(name="outp", bufs=4))
        weffp = ctx.enter_context(tc.tile_pool(name="weffp", bufs=2))
        slabp = ctx.enter_context(tc.tile_pool(name="slabp", bufs=4))
        pp = ctx.enter_context(tc.tile_pool(name="pp", bufs=3, space="PSUM"))
        pt = ctx.enter_context(tc.tile_pool(name="pt", bufs=2, space="PSUM"))
        pa = ctx.enter_context(tc.tile_pool(name="pa", bufs=2, space="PSUM"))
        ps = ctx.enter_context(tc.tile_pool(name="ps", bufs=1, space="PSUM"))

        ident_f = singles.tile([128, 128], F32, name="ident_f")
        make_identity(nc, ident_f)
        ident_b = singles.tile([128, 128], BF16, name="ident_b")
        make_identity(nc, ident_b)
        ones = singles.tile([128, 1], F32, name="ones")
        nc.vector.memset(ones, 1.0)
        rkmat = singles.tile([128, 128], F32, name="rkmat")
        nc.vector.memset(rkmat, 0.0)
        epsin = singles.tile([1, 1], F32, name="epsin")
        nc.vector.memset(epsin, EPS_IN)

        wpo_sb = singles.tile([128, 4, 128], F32, name="wpo_sb")

        qT = singles.tile([128, HW], BF16, name="qT")
        kvp = ctx.enter_context(tc.tile_pool(name="kvp", bufs=2))
        qss = singles.tile([128, 32], F32, name="qss")
        kss = singles.tile([128, 32], F32, name="kss")
        rqs = singles.tile([128, 1], F32, name="rqs")
        rqs2 = singles.tile([128, 1], F32, name="rqs2")

        # ---------------- Q phase (emitted interleaved with conv(0)) ----------
        qpool = ctx.enter_context(tc.tile_pool(name="qpool", bufs=1))
        wqp = ctx.enter_context(tc.tile_pool(name="wqp", bufs=3))
        textT_sb = qpool.tile([128, KP, 128], BF16, name="textT_sb")
        nc.sync.dma_start(out=textT_sb,
                          in_=textT_d.rearrange("(k p) c -> p k c", p=128))
        wq_r = wqT_d.rearrange("(k p) n -> p k n", p=128)

        def q_chunk(j):
            wq_t = wqp.tile([128, KP, 512], BF16, name="wq_t", tag="wq_t")
            nc.sync.dma_start(out=wq_t, in_=wq_r[:, :, j * 512:(j + 1) * 512])
            psq = pp.tile([128, 512], F32, name="psq", tag="pp")
            for k in range(KP):
                nc.tensor.matmul(psq,
                                 textT_sb[:, k, :],
                                 wq_t[:, k, :],
                                 start=(k == 0), stop=(k == KP - 1))
            stgq = stgp.tile([128, 512], BF16, name="stgq", tag="stg")
            nc.vector.tensor_copy(out=stgq, in_=psq)
            sqo = scratch.tile([128, 512], BF16, name="sqo", tag="sqo")
            nc.vector.tensor_mul(sqo, stgq, stgq)
            nc.vector.tensor_reduce(out=qss[:, j:j + 1], in_=sqo,
                                    axis=AX.X, op=ALU.add)
            for b4 in range(4):
                blk = 4 * j + b4
                ptt = pt.tile([128, 128], BF16, name="ptt", tag="pt")
                nc.tensor.transpose(ptt, stgq[:, b4 * 128:(b4 + 1) * 128],
                                    ident_b)
                dst = qT[:, blk * 128:(blk + 1) * 128]
                if blk % 2 == 0:
                    nc.vector.tensor_copy(out=dst, in_=ptt)
                else:
                    nc.scalar.copy(out=dst, in_=ptt)

        def q_finalize():
            qn = small.tile([128, 1], F32, name="qn")
            nc.vector.tensor_reduce(out=qn, in_=qss, axis=AX.X, op=ALU.add)
            nc.scalar.sqrt(qn, qn)
            nc.vector.tensor_scalar_max(qn, qn, EPS_NORM)
            rq = small.tile([128, 1], F32, name="rq")
            nc.vector.reciprocal(rq, qn)
            nc.scalar.mul(rqs, rq, RSQRT_C)
            nc.vector.tensor_mul(rqs2, rqs, rqs)

        # ---------------- branches (software-pipelined) ----------------
        def prefetch_branch(i):
            wk_sb = weffp.tile([128, 9, 128], BF16, name=f"wk{i}", tag="wk")
            nc.sync.dma_start(out=wk_sb, in_=weffk_d[i].rearrange("t c o -> c t o"))
            wv_sb = weffp.tile([128, 9, 128], BF16, name=f"wv{i}", tag="wv")
            nc.sync.dma_start(out=wv_sb, in_=weffv_d[i].rearrange("t c o -> c t o"))
            slab0 = slabp.tile([128, 10, 128], BF16, name=f"slab{i}_0", tag="slab")
            emb_r = embs_d[i].rearrange("c (h w) -> c h w", w=128)
            nc.sync.dma_start(out=slab0[:, 0:9, :], in_=emb_r[:, 0:9, :])
            return (wk_sb, wv_sb, slab0)

        def conv_phase(i, pre, slab_hook=None, mid_hook=None, stage_hook=None,
                       pre_hook=None, do_k=True, do_v=True, st=None):
            if st is None:
                st = {"i": i}
            if pre is not None:
                wk_sb, wv_sb, slab0 = pre
                st["wk_sb"], st["wv_sb"] = wk_sb, wv_sb
            else:
                wk_sb, wv_sb = st["wk_sb"], st["wv_sb"]
                slab0 = None
            emb_r = embs_d[i].rearrange("c (h w) -> c h w", w=128)
            if do_k:
                pattn = pa.tile([128, 128], F32, name=f"pattn{i}", tag="pa")
                kssb = kvp.tile([128, 32], F32, name=f"kss{i}", tag="kss")
                st["pattn"], st["kssb"] = pattn, kssb
            else:
                pattn, kssb = st["pattn"], st["kssb"]
            if do_v:
                vv = kvp.tile([128, HW], BF16, name=f"vv{i}", tag="vv")
                st["vv"] = vv
            vv = st.get("vv")
            for s in range(16):
                if slab_hook is not None:
                    slab_hook(s)
                if stage_hook is not None:
                    stage_hook(s)
                if s == 13 and pre_hook is not None:
                    pre_hook()
                # image rows [a, b) of the input; no zero padding — pad taps
                # simply skip out-of-range rows/cols (first tap writing an
                # element overwrites, so partial-coverage taps are exact).
                if s == 0 and slab0 is not None:
                    slab = slab0
                else:
                    slab = slabp.tile([128, 10, 128], BF16,
                                      name=f"slab{i}_{s}_{do_k}", tag="slab")
                    a = max(8 * s - 1, 0)
                    b_ = min(8 * s + 9, 128)
                    nc.sync.dma_start(out=slab[:, 0:(b_ - a), :],
                                      in_=emb_r[:, a:b_, :])

                def conv_chunk(psum, w_sb, h2):
                    # center tap (dy=1,dx=1) first: it always covers the full
                    # chunk, so start=True initializes every psum element.
                    order = [4, 0, 1, 2, 3, 5, 6, 7, 8]
                    for n_t, t in enumerate(order):
                        dy, dx = TAPS[t]
                        rr0 = 1 if (s == 0 and h2 == 0 and dy == 0) else 0
                        rr1 = 3 if (s == 15 and h2 == 1 and dy == 2) else 4
                        base = 4 * h2 + dy - (1 if s == 0 else 0)
                        co0, co1 = (1, 128) if dx == 0 else (0, 127) if dx == 2 else (0, 128)
                        ci0, ci1 = (0, 127) if dx == 0 else (1, 128) if dx == 2 else (0, 128)
                        nc.tensor.matmul(psum[:, rr0:rr1, co0:co1],
                                         w_sb[:, t, :],
                                         slab[:, base + rr0:base + rr1, ci0:ci1],
                                         start=(n_t == 0), stop=(n_t == 8),
                                         skip_group_check=True)

                for h2 in range(2):
                    j = 2 * s + h2
                    if not do_k:
                        psv = pp.tile([128, 4, 128], F32, name="psv", tag="pp")
                        conv_chunk(psv, wv_sb, h2)
                        nc.scalar.copy(out=vv[:, j * 512:(j + 1) * 512],
                                       in_=psv.rearrange("p a b -> p (a b)"))
                        continue
                    psk = pp.tile([128, 4, 128], F32, name="psk", tag="pp")
                    conv_chunk(psk, wk_sb, h2)
                    stgk = stgp.tile([128, 512], BF16, name="stgk", tag="stg")
                    nc.vector.tensor_copy(out=stgk,
                                          in_=psk.rearrange("p a b -> p (a b)"))
                    sqo = scratch.tile([128, 512], BF16, name="sqo", tag="sqo")
                    nc.vector.tensor_mul(sqo, stgk, stgk)
                    nc.vector.tensor_reduce(out=kssb[:, j:j + 1], in_=sqo,
                                            axis=AX.X, op=ALU.add)
                    # transpose each 128-block and accumulate attn inline
                    for b4 in range(4):
                        jj = 4 * j + b4
                        ptt = pt.tile([128, 128], BF16, name="ptk", tag="pt")
                        nc.tensor.transpose(ptt, stgk[:, b4 * 128:(b4 + 1) * 128],
                                            ident_b)
                        ktb = stgp.tile([128, 128], BF16, name="ktb", tag="ktb")
                        if jj % 2 == 0:
                            nc.vector.tensor_copy(out=ktb, in_=ptt)
                        else:
                            nc.scalar.copy(out=ktb, in_=ptt)
                        nc.tensor.matmul(pattn,
                                         qT[:, jj * 128:(jj + 1) * 128], ktb,
                                         start=(jj == 0), stop=(jj == 127),
                                         skip_group_check=True)
                    if do_v:
                        psv = pp.tile([128, 4, 128], F32, name="psv", tag="pp")
                        conv_chunk(psv, wv_sb, h2)
                        nc.scalar.copy(out=vv[:, j * 512:(j + 1) * 512],
                                       in_=psv.rearrange("p a b -> p (a b)"))
            return st

        def finish_a(st):
            i, pattn, kssb = st["i"], st["pattn"], st["kssb"]
            # kk row norms -> rk, transposed into a broadcast row
            kn = small.tile([128, 1], F32, name="kn")
            nc.vector.tensor_reduce(out=kn, in_=kssb, axis=AX.X, op=ALU.add)
            nc.scalar.sqrt(kn, kn)
            nc.vector.tensor_scalar_max(kn, kn, EPS_NORM)
            rk = small.tile([128, 1], F32, name="rk")
            nc.vector.reciprocal(rk, kn)
            nc.vector.tensor_copy(out=rkmat[:, 0:1], in_=rk)
            psrk = ps.tile([128, 128], F32, name="psrk", tag="ps")
            nc.tensor.transpose(psrk, rkmat, ident_f)
            rkrow = small.tile([1, 128], F32, name="rkrow")
            nc.vector.tensor_copy(out=rkrow, in_=psrk[0:1, :])
            rkfull = med.tile([128, 128], F32, name="rkfull", tag="rkfull")
            nc.gpsimd.partition_broadcast(rkfull, rkrow)

            # z1 = attn_raw * rk[e];  stats of attn_s = z1 * rqs[c]
            z1 = med.tile([128, 128], F32, name="z1", tag="z1")
            nc.vector.tensor_mul(z1, pattn, rkfull)
            rs = small.tile([128, 1], F32, name="rs")
            nc.vector.tensor_reduce(out=rs, in_=z1, axis=AX.X, op=ALU.add)
            rs_s = small.tile([128, 1], F32, name="rs_s")
            nc.vector.tensor_mul(rs_s, rs, rqs)
            sq2 = scratch.tile([128, 128], F32, name="sq2", tag="sq")
            rss = small.tile([128, 1], F32, name="rss")
            nc.scalar.activation(out=sq2, in_=z1, func=ACTF.Square, accum_out=rss)
            rss_s = small.tile([128, 1], F32, name="rss_s")
            nc.vector.tensor_mul(rss_s, rss, rqs2)
            st2 = small.tile([128, 2], F32, name="st2")
            nc.vector.tensor_copy(out=st2[:, 0:1], in_=rs_s)
            nc.vector.tensor_copy(out=st2[:, 1:2], in_=rss_s)
            psst = ps.tile([1, 2], F32, name="psst", tag="ps")
            nc.tensor.matmul(psst, ones, st2)
            mu = small.tile([1, 1], F32, name="mu")
            nc.scalar.mul(mu, psst[0:1, 0:1], 1.0 / (C * C))
            ms = small.tile([1, 1], F32, name="ms")
            nc.scalar.mul(ms, psst[0:1, 1:2], 1.0 / (C * C))
            mu2 = small.tile([1, 1], F32, name="mu2")
            nc.scalar.square(mu2, mu)
            var = small.tile([1, 1], F32, name="var")
            nc.vector.tensor_sub(var, ms, mu2)
            std = small.tile([1, 1], F32, name="std")
            nc.scalar.activation(out=std, in_=var, func=ACTF.Sqrt, bias=epsin)
            rstd1 = small.tile([1, 1], F32, name="rstd1")
            nc.vector.reciprocal(rstd1, std)
            rstdf = small.tile([128, 1], F32, name="rstdf")
            nc.gpsimd.partition_broadcast(rstdf, rstd1)
            # softmax over e of z1*scale_c (instance-norm mean shift cancels)
            scale_c = small.tile([128, 1], F32, name="scale_c")
            nc.vector.tensor_mul(scale_c, rqs, rstdf)
            rm = small.tile([128, 1], F32, name="rm")
            nc.vector.tensor_reduce(out=rm, in_=z1, axis=AX.X, op=ALU.max)
            be = small.tile([128, 1], F32, name="be")
            nc.vector.tensor_mul(be, scale_c, rm)
            be_n = small.tile([128, 1], F32, name="be_n")
            nc.scalar.mul(be_n, be, -1.0)
            expb = med.tile([128, 128], F32, name="expb", tag="expb")
            se = small.tile([128, 1], F32, name="se")
            nc.scalar.activation(out=expb, in_=z1, func=ACTF.Exp,
                                 scale=scale_c, bias=be_n, accum_out=se)
            rse = small.tile([128, 1], F32, name="rse")
            nc.vector.reciprocal(rse, se)
            p_sb = med.tile([128, 128], F32, name="p_sb", tag="p_sb")
            nc.scalar.mul(p_sb, expb, rse)
            st["p_sb"] = p_sb

        def finish_b(st):
            i, p_sb = st["i"], st["p_sb"]
            # P2T = p.T @ Wpo_i.T  -> lhsT for the output matmul
            psp2 = ps.tile([128, 128], F32, name="psp2", tag="ps")
            nc.tensor.matmul(psp2, p_sb, wpo_sb[:, i, :])
            p2t = med.tile([128, 128], BF16, name="p2t", tag="p2t")
            nc.vector.tensor_copy(out=p2t, in_=psp2)
            st["p2t"] = p2t

        def finish_c(st, j0, j1):
            i, p2t, vv = st["i"], st["p2t"], st["vv"]
            for j in range(j0, j1):
                pso = pp.tile([128, 512], F32, name="pso", tag="pp")
                nc.tensor.matmul(pso, p2t, vv[:, j * 512:(j + 1) * 512])
                oc = outp.tile([128, 512], F32, name="oc", tag="oc")
                if j % 2 == 0:
                    nc.vector.tensor_copy(out=oc, in_=pso)
                else:
                    nc.scalar.copy(out=oc, in_=pso)
                nc.sync.dma_start(out=outs_d[i][:, j * 512:(j + 1) * 512], in_=oc)

        prev = [None]

        def stage_hook(s):
            st = prev[0]
            if st is None:
                return
            if s == 2:
                finish_a(st)
            elif s == 3:
                finish_b(st)
            elif s >= 4:
                j0 = st.get("j", 0)
                j1 = min(j0 + 3, 32)
                if j0 < j1:
                    finish_c(st, j0, j1)
                    st["j"] = j1

        def q_slab_hook(s):
            if s < 15:
                q_chunk(2 * s + 2)
                q_chunk(2 * s + 3)
            else:
                q_finalize()

        q_chunk(0)
        q_chunk(1)
        pre_state = {0: prefetch_branch(0)}
        nc.sync.dma_start(out=wpo_sb, in_=wpoT_d.rearrange("i c o -> c i o"))

        def make_pre_hook(nxt):
            def hook():
                pre_state[nxt] = prefetch_branch(nxt)
            return hook

        for i in range(3):
            state = conv_phase(i, pre_state.pop(i),
                               slab_hook=(q_slab_hook if i == 0 else None),
                               mid_hook=None,
                               stage_hook=stage_hook,
                               pre_hook=make_pre_hook(i + 1) if i < 3 else None)
            prev[0] = state
        # branch 3: k-pass (with attn) first, so the softmax chain and the
        # output matmuls overlap the v-pass instead of trailing the kernel.
        st3 = conv_phase(3, pre_state.pop(3), stage_hook=stage_hook,
                         do_v=False)
        prev[0] = None
        finish_a(st3)
        finish_b(st3)

        def pso_hook(s):
            j0 = st3.get("j", 0)
            j1 = min(max(2 * s - 2, 0), 32)
            if j0 < j1:
                finish_c(st3, j0, j1)
                st3["j"] = j1

        conv_phase(3, None, stage_hook=pso_hook, do_k=False, st=st3)
        finish_c(st3, st3.get("j", 0), 32)

def _build_nc():
    nc = bacc.Bacc("TRN2", target_bir_lowering=False, debug=False, num_devices=8)
    textT_d = nc.dram_tensor("textT", [KP * 128, C], BF16, kind="ExternalInput")
    wqT_d = nc.dram_tensor("wqT", [KP * 128, HW], BF16, kind="ExternalInput")
    embs_d = [nc.dram_tensor(f"emb{i}", [C, HW], BF16, kind="ExternalInput")
              for i in range(4)]
    weffk_d = nc.dram_tensor("weffk", [4, 9, C, C], BF16, kind="ExternalInput")
    weffv_d = nc.dram_tensor("weffv", [4, 9, C, C], BF16, kind="ExternalInput")
    wpoT_d = nc.dram_tensor("wpoT", [4, C, C], F32, kind="ExternalInput")
    outs_d = [nc.dram_tensor(f"out{i}", [C, HW], F32, kind="ExternalOutput")
              for i in range(4)]
    with tile.TileContext(nc) as tc:
        _body(nc, tc, textT_d, wqT_d, embs_d, weffk_d, weffv_d, wpoT_d, outs_d)
    nc.compile()
    return nc


_NC = None


def _get_nc():
    global _NC
    if _NC is None:
        _NC = _build_nc()
    return _NC


def _prep_in_maps(emb1, emb2, emb3, emb4, text_emb, Wq, bq, Wmk, Wk, Wmv, Wv, Wpo):
    f32 = np.float32
    embs = [np.ascontiguousarray(np.asarray(e, f32).reshape(B, C, HW))
            for e in (emb1, emb2, emb3, emb4)]
    text_emb = np.asarray(text_emb, f32)
    Wq = np.asarray(Wq, f32)
    bq = np.asarray(bq, f32)
    Wmk = np.asarray(Wmk, f32)
    Wk = np.asarray(Wk, f32)
    Wmv = np.asarray(Wmv, f32)
    Wv = np.asarray(Wv, f32)
    Wpo = np.asarray(Wpo, f32)

    wqT = np.zeros((KP * 128, HW), f32)
    wqT[:TS] = Wq.T
    wqT[TS] = bq
    wqT = wqT.astype(BF16_NP)

    g2 = (np.arange(C) // 2) * 2

    def build_weff(Wm, Wg):
        out = np.empty((4, 9, C, C), f32)
        for i in range(4):
            A = Wg[i][:, 0].reshape(C, 9)
            Bt = Wg[i][:, 1].reshape(C, 9)
            M0 = Wm[i][g2, :]
            M1 = Wm[i][g2 + 1, :]
            out[i] = (np.einsum('ot,oc->tco', A, M0)
                      + np.einsum('ot,oc->tco', Bt, M1)).astype(f32)
        return np.ascontiguousarray(out.astype(BF16_NP))

    weffk = build_weff(Wmk, Wk)
    weffv = build_weff(Wmv, Wv)
    wpoT = np.ascontiguousarray(np.transpose(Wpo, (0, 2, 1)))

    in_maps = []
    for b in range(B):
        textT = np.zeros((KP * 128, C), f32)
        textT[:TS] = text_emb[b, 0].T
        textT[TS] = 1.0
        textT = textT.astype(BF16_NP)
        m = {"textT": textT, "wqT": wqT, "weffk": weffk, "weffv": weffv,
             "wpoT": wpoT}
        for i in range(4):
            m[f"emb{i}"] = np.ascontiguousarray(embs[i][b].astype(BF16_NP))
        in_maps.append(m)
    return in_maps


def _run(in_maps, trace=False):
    nc = _get_nc()
    return bass_utils.run_bass_kernel_spmd(nc, in_maps, core_ids=list(range(8)),
                                           trace=trace)


def kernel(emb1, emb2, emb3, emb4, text_emb, Wq, bq, Wmk, Wk, Wmv, Wv, Wpo):
    in_maps = _prep_in_maps(emb1, emb2, emb3, emb4, text_emb, Wq, bq,
                            Wmk, Wk, Wmv, Wv, Wpo)
    res = _run(in_maps, trace=False)
    outs = []
    for i in range(4):
        o = np.stack([res.results[b][f"out{i}"].reshape(C, H, W)
                      for b in range(B)])
        outs.append(np.ascontiguousarray(o.astype(np.float32)))
    return tuple(outs)



# revision 2
# speedup vs baseline: 1.1453x; 1.1453x over previous
"""Trainium2 Bass kernel for the Dblock-ViT channel-attention module.

Strategy: data-parallel over batch (8 batches -> 8 NeuronCores). Each core:
  q       = l2norm(text_emb[b] @ Wq.T + bq)              [C, HW]
  per branch i in 0..3:
    kk    = fused dense 3x3 conv (1x1 folded into taps)  [C, HW]
    vv    = same for the v path                          [C, HW]
    attn  = instancenorm(q_n @ kk_n.T / sqrt(C)); p = softmax rows
    out_i = (Wpo_i @ p) @ vv                             [C, HW]

The k-path conv runs in fp8-e4m3 with DoubleRow perf mode (2 taps per
matmul): 4 DR matmuls + 1 single per 512-wide chunk instead of 9 bf16
matmuls. The k conv output feeds l2norm, so the fp8 weight scaling
cancels and quantization noise is strongly attenuated by the softmax.
The v-path conv stays bf16 (its error passes linearly to the output).
The fp8 slab is zero-padded to 130 columns (and one halo row top or
bottom) so every tap runs full-width with no edge-range special cases.
"""

import math
import sys
import types

import ml_dtypes
import numpy as np

BF16_NP = ml_dtypes.bfloat16
E4_NP = ml_dtypes.float8_e4m3

for _p in ("/opt/trn_rl_repo",):
    if _p not in sys.path:
        sys.path.insert(0, _p)

# The image's antenv package lacks axon_hooks; register a functional stand-in
# so run_bass_kernel_spmd(trace=True) can reach the NTFF profiling hook.
try:
    import antenv
    if "antenv.axon_hooks" not in sys.modules:
        _m = types.ModuleType("antenv.axon_hooks")
        _m._hook_val = None
        _m.set_axon_ntff_profile_hook = lambda h: setattr(_m, "_hook_val", h)
        _m.get_axon_ntff_profile_hook = lambda: _m._hook_val
        sys.modules["antenv.axon_hooks"] = _m
        antenv.axon_hooks = _m
        try:
            from trn_agent_boot.trn_boot import _ntff_profile_via_ctypes
            _m._hook_val = _ntff_profile_via_ctypes("/opt/axon/libaxon_pjrt.so")
        except Exception:
            pass
except Exception:
    pass

import concourse.bass as bass
import concourse.mybir as mybir
import concourse.tile as tile
from concourse import bacc, bass_utils
from concourse.ap import AP as BassAP
from concourse.masks import make_identity

try:
    bass_utils.upload_artifacts = lambda tmpdir: tmpdir
except Exception:
    pass

B, C, H, W = 8, 128, 128, 128
HW = H * W
TS = 512
EPS_NORM = 1e-12
EPS_IN = 1e-5
RSQRT_C = 1.0 / math.sqrt(C)
F32 = mybir.dt.float32
F32R = mybir.dt.float32r
BF16 = mybir.dt.bfloat16
FP8 = mybir.dt.float8e4
DRM = mybir.MatmulPerfMode.DoubleRow
TAPS = [(dy, dx) for dy in range(3) for dx in range(3)]
# DoubleRow tap order: 3 dx-pairs (stride 2), 1 dy-pair (stride 260), 1 single
DR_ORDER = [0, 2, 3, 5, 6, 8, 1, 7, 4]
KSL = 1300  # fp8 slab partition stride: 10 rows x 130 cols
AX = mybir.AxisListType
ALU = mybir.AluOpType
ACTF = mybir.ActivationFunctionType


def _body(nc, tc, kp, textT_d, wqT_d, embs_d, embs8_d, weffk8_d, weffv_d,
          wpoT_d, outs_d):
    from contextlib import ExitStack
    ctx = ExitStack()
    with ctx:
        singles = ctx.enter_context(tc.tile_pool(name="singles", bufs=1))
        small = ctx.enter_context(tc.tile_pool(name="small", bufs=1))
        med = ctx.enter_context(tc.tile_pool(name="med", bufs=2))
        scratch = ctx.enter_context(tc.tile_pool(name="scratch", bufs=2))
        stgp = ctx.enter_context(tc.tile_pool(name="stgp", bufs=3))
        outp = ctx.enter_context(tc.tile_pool(name="outp", bufs=4))
        weffp = ctx.enter_context(tc.tile_pool(name="weffp", bufs=2))
        slabp = ctx.enter_context(tc.tile_pool(name="slabp", bufs=4))
        slab8p = ctx.enter_context(tc.tile_pool(name="slab8p", bufs=4))
        pp = ctx.enter_context(tc.tile_pool(name="pp", bufs=3, space="PSUM"))
        pt = ctx.enter_context(tc.tile_pool(name="pt", bufs=2, space="PSUM"))
        pa = ctx.enter_context(tc.tile_pool(name="pa", bufs=2, space="PSUM"))
        ps = ctx.enter_context(tc.tile_pool(name="ps", bufs=1, space="PSUM"))

        ident_f = singles.tile([128, 128], F32, name="ident_f")
        make_identity(nc, ident_f)
        ident_b = singles.tile([128, 128], BF16, name="ident_b")
        make_identity(nc, ident_b)
        ones = singles.tile([128, 1], F32, name="ones")
        nc.vector.memset(ones, 1.0)
        rkmat = singles.tile([128, 128], F32, name="rkmat")
        nc.vector.memset(rkmat, 0.0)
        epsin = singles.tile([1, 1], F32, name="epsin")
        nc.vector.memset(epsin, EPS_IN)

        wpo_sb = singles.tile([128, 4, 128], F32, name="wpo_sb")

        qT = singles.tile([128, HW], BF16, name="qT")
        kvp = ctx.enter_context(tc.tile_pool(name="kvp", bufs=2))
        qss = singles.tile([128, 32], F32, name="qss")
        rqs = singles.tile([128, 1], F32, name="rqs")
        rqs2 = singles.tile([128, 1], F32, name="rqs2")

        # fp8 slab buffers: zero the padding columns once; the loop DMAs only
        # write cols 1..128 so the zero columns persist across buffer reuse.
        for _zi in range(4):
            zt = slab8p.tile([128, 10, 130], FP8, name=f"kz{_zi}", tag="kslab")
            nc.vector.memset(zt[:, :, 0:1], 0.0)
            nc.vector.memset(zt[:, :, 129:130], 0.0)

        # ---------------- Q phase (emitted interleaved with conv(0)) ----------
        qpool = ctx.enter_context(tc.tile_pool(name="qpool", bufs=1))
        wqp = ctx.enter_context(tc.tile_pool(name="wqp", bufs=3))
        textT_sb = qpool.tile([128, kp, 128], BF16, name="textT_sb")
        nc.sync.dma_start(out=textT_sb,
                          in_=textT_d.rearrange("(k p) c -> p k c", p=128))
        wq_r = wqT_d.rearrange("(k p) n -> p k n", p=128)

        def q_chunk(j):
            wq_t = wqp.tile([128, kp, 512], BF16, name="wq_t", tag="wq_t")
            nc.sync.dma_start(out=wq_t, in_=wq_r[:, :, j * 512:(j + 1) * 512])
            psq = pp.tile([128, 512], F32, name="psq", tag="pp")
            for k in range(kp):
                nc.tensor.matmul(psq,
                                 textT_sb[:, k, :],
                                 wq_t[:, k, :],
                                 start=(k == 0), stop=(k == kp - 1))
            stgq = stgp.tile([128, 512], BF16, name="stgq", tag="stg")
            nc.vector.tensor_copy(out=stgq, in_=psq)
            sqo = scratch.tile([128, 512], BF16, name="sqo", tag="sqo")
            nc.vector.tensor_mul(sqo, stgq, stgq)
            nc.vector.tensor_reduce(out=qss[:, j:j + 1], in_=sqo,
                                    axis=AX.X, op=ALU.add)
            for b4 in range(4):
                blk = 4 * j + b4
                ptt = pt.tile([128, 128], BF16, name="ptt", tag="pt")
                nc.tensor.transpose(ptt, stgq[:, b4 * 128:(b4 + 1) * 128],
                                    ident_b)
                dst = qT[:, blk * 128:(blk + 1) * 128]
                if blk % 2 == 0:
                    nc.vector.tensor_copy(out=dst, in_=ptt)
                else:
                    nc.scalar.copy(out=dst, in_=ptt)

        def q_finalize():
            qn = small.tile([128, 1], F32, name="qn")
            nc.vector.tensor_reduce(out=qn, in_=qss, axis=AX.X, op=ALU.add)
            nc.scalar.sqrt(qn, qn)
            nc.vector.tensor_scalar_max(qn, qn, EPS_NORM)
            rq = small.tile([128, 1], F32, name="rq")
            nc.vector.reciprocal(rq, qn)
            nc.scalar.mul(rqs, rq, RSQRT_C)
            nc.vector.tensor_mul(rqs2, rqs, rqs)

        # ---------------- slab loading ----------------
        def load_kslab(i, s):
            kslab = slab8p.tile([128, 10, 130], FP8, name=f"ks{i}_{s}",
                                tag="kslab")
            emb8_r = embs8_d[i].rearrange("c (h w) -> c h w", w=128)
            if s == 0:
                nc.vector.memset(kslab[:, 0:1, :], 0.0)
                nc.sync.dma_start(out=kslab[:, 1:10, 1:129],
                                  in_=emb8_r[:, 0:9, :])
            elif s == 15:
                nc.vector.memset(kslab[:, 9:10, :], 0.0)
                nc.sync.dma_start(out=kslab[:, 0:9, 1:129],
                                  in_=emb8_r[:, 119:128, :])
            else:
                nc.sync.dma_start(out=kslab[:, 0:10, 1:129],
                                  in_=emb8_r[:, 8 * s - 1:8 * s + 9, :])
            return kslab

        def kconv_chunk(psk, wk_sb, kslab, h2):
            # 4 DoubleRow matmuls (2 taps each) + 1 single; all full 512-wide.
            def rhs(off, slot=None):
                dims = [(KSL, 128)]
                if slot is not None:
                    dims.append((slot, 2))
                dims += [(130, 4), (1, 128)]
                return BassAP(kslab.tensor, kslab.offset + off, dims)
            base = 520 * h2
            nc.tensor.matmul(psk, wk_sb[:, 0:2, :], rhs(base, 2),
                             start=True, stop=False, perf_mode=DRM,
                             skip_group_check=True)
            nc.tensor.matmul(psk, wk_sb[:, 2:4, :], rhs(base + 130, 2),
                             start=False, stop=False, perf_mode=DRM,
                             skip_group_check=True)
            nc.tensor.matmul(psk, wk_sb[:, 4:6, :], rhs(base + 260, 2),
                             start=False, stop=False, perf_mode=DRM,
                             skip_group_check=True)
            nc.tensor.matmul(psk, wk_sb[:, 6:8, :], rhs(base + 1, 260),
                             start=False, stop=False, perf_mode=DRM,
                             skip_group_check=True)
            nc.tensor.matmul(psk, wk_sb[:, 8, :], rhs(base + 131),
                             start=False, stop=True, skip_group_check=True)

        # ---------------- branches (software-pipelined) ----------------
        def prefetch_branch(i, k=True, v=True):
            out = {}
            if k:
                wk_sb = weffp.tile([128, 9, 128], FP8, name=f"wk{i}", tag="wk")
                nc.sync.dma_start(out=wk_sb,
                                  in_=weffk8_d[i].rearrange("t c o -> c t o"))
                out["wk_sb"] = wk_sb
                out["kslab0"] = load_kslab(i, 0)
            if v:
                wv_sb = weffp.tile([128, 9, 128], BF16, name=f"wv{i}", tag="wv")
                nc.sync.dma_start(out=wv_sb,
                                  in_=weffv_d[i].rearrange("t c o -> c t o"))
                out["wv_sb"] = wv_sb
                slab0 = slabp.tile([128, 10, 128], BF16, name=f"slab{i}_0",
                                   tag="slab")
                emb_r = embs_d[i].rearrange("c (h w) -> c h w", w=128)
                nc.sync.dma_start(out=slab0[:, 0:9, :], in_=emb_r[:, 0:9, :])
                out["slab0"] = slab0
            return out

        def conv_phase(i, pre, slab_hook=None, stage_hook=None,
                       pre_hook=None, do_k=True, do_v=True, st=None):
            if st is None:
                st = {"i": i}
            if pre is not None:
                st.update(pre)
            wk_sb = st.get("wk_sb")
            wv_sb = st.get("wv_sb")
            emb_r = embs_d[i].rearrange("c (h w) -> c h w", w=128)
            if do_k:
                pattn = pa.tile([128, 128], F32, name=f"pattn{i}", tag="pa")
                kssb = kvp.tile([128, 32], F32, name=f"kss{i}", tag="kss")
                st["pattn"], st["kssb"] = pattn, kssb
            else:
                pattn, kssb = st["pattn"], st["kssb"]
            if do_v:
                vv = kvp.tile([128, HW], BF16, name=f"vv{i}", tag="vv")
                st["vv"] = vv
            vv = st.get("vv")
            for s in range(16):
                if slab_hook is not None:
                    slab_hook(s)
                if stage_hook is not None:
                    stage_hook(s)
                if s == 13 and pre_hook is not None:
                    pre_hook()
                if do_k:
                    if s == 0:
                        kslab = st.pop("kslab0")
                    else:
                        kslab = load_kslab(i, s)
                if do_v:
                    if s == 0 and "slab0" in st:
                        slab = st.pop("slab0")
                    else:
                        slab = slabp.tile([128, 10, 128], BF16,
                                          name=f"slab{i}_{s}_{do_k}",
                                          tag="slab")
                        a = max(8 * s - 1, 0)
                        b_ = min(8 * s + 9, 128)
                        nc.sync.dma_start(out=slab[:, 0:(b_ - a), :],
                                          in_=emb_r[:, a:b_, :])

                def vconv_chunk(psum, w_sb, h2):
                    # center tap (dy=1,dx=1) first: it always covers the full
                    # chunk, so start=True initializes every psum element.
                    order = [4, 0, 1, 2, 3, 5, 6, 7, 8]
                    for n_t, t in enumerate(order):
                        dy, dx = TAPS[t]
                        rr0 = 1 if (s == 0 and h2 == 0 and dy == 0) else 0
                        rr1 = 3 if (s == 15 and h2 == 1 and dy == 2) else 4
                        base = 4 * h2 + dy - (1 if s == 0 else 0)
                        co0, co1 = (1, 128) if dx == 0 else (0, 127) if dx == 2 else (0, 128)
                        ci0, ci1 = (0, 127) if dx == 0 else (1, 128) if dx == 2 else (0, 128)
                        nc.tensor.matmul(psum[:, rr0:rr1, co0:co1],
                                         w_sb[:, t, :],
                                         slab[:, base + rr0:base + rr1, ci0:ci1],
                                         start=(n_t == 0), stop=(n_t == 8),
                                         skip_group_check=True)

                for h2 in range(2):
                    j = 2 * s + h2
                    if not do_k:
                        psv = pp.tile([128, 4, 128], F32, name="psv", tag="pp")
                        vconv_chunk(psv, wv_sb, h2)
                        nc.scalar.copy(out=vv[:, j * 512:(j + 1) * 512],
                                       in_=psv.rearrange("p a b -> p (a b)"))
                        continue
                    psk = pp.tile([128, 512], F32, name="psk", tag="pp")
                    kconv_chunk(psk, wk_sb, kslab, h2)
                    stgk = stgp.tile([128, 512], BF16, name="stgk", tag="stg")
                    nc.vector.tensor_copy(out=stgk, in_=psk)
                    sqo = scratch.tile([128, 512], BF16, name="sqo", tag="sqo")
                    nc.vector.tensor_mul(sqo, stgk, stgk)
                    nc.vector.tensor_reduce(out=kssb[:, j:j + 1], in_=sqo,
                                            axis=AX.X, op=ALU.add)
                    # transpose each 128-block and accumulate attn inline
                    for b4 in range(4):
                        jj = 4 * j + b4
                        ptt = pt.tile([128, 128], BF16, name="ptk", tag="pt")
                        nc.tensor.transpose(ptt, stgk[:, b4 * 128:(b4 + 1) * 128],
                                            ident_b)
                        ktb = stgp.tile([128, 128], BF16, name="ktb", tag="ktb")
                        if jj % 2 == 0:
                            nc.vector.tensor_copy(out=ktb, in_=ptt)
                        else:
                            nc.scalar.copy(out=ktb, in_=ptt)
                        nc.tensor.matmul(pattn,
                                         qT[:, jj * 128:(jj + 1) * 128], ktb,
                                         start=(jj == 0), stop=(jj == 127),
                                         skip_group_check=True)
                    if do_v:
                        psv = pp.tile([128, 4, 128], F32, name="psv", tag="pp")
                        vconv_chunk(psv, wv_sb, h2)
                        nc.scalar.copy(out=vv[:, j * 512:(j + 1) * 512],
                                       in_=psv.rearrange("p a b -> p (a b)"))
            return st

        def finish_a(st):
            i, pattn, kssb = st["i"], st["pattn"], st["kssb"]
            # kk row norms -> rk, transposed into a broadcast row
            kn = small.tile([128, 1], F32, name="kn")
            nc.vector.tensor_reduce(out=kn, in_=kssb, axis=AX.X, op=ALU.add)
            nc.scalar.sqrt(kn, kn)
            nc.vector.tensor_scalar_max(kn, kn, EPS_NORM)
            rk = small.tile([128, 1], F32, name="rk")
            nc.vector.reciprocal(rk, kn)
            nc.vector.tensor_copy(out=rkmat[:, 0:1], in_=rk)
            psrk = ps.tile([128, 128], F32, name="psrk", tag="ps")
            nc.tensor.transpose(psrk, rkmat, ident_f)
            rkrow = small.tile([1, 128], F32, name="rkrow")
            nc.vector.tensor_copy(out=rkrow, in_=psrk[0:1, :])
            rkfull = med.tile([128, 128], F32, name="rkfull", tag="rkfull")
            nc.gpsimd.partition_broadcast(rkfull, rkrow)

            # z1 = attn_raw * rk[e];  stats of attn_s = z1 * rqs[c]
            z1 = med.tile([128, 128], F32, name="z1", tag="z1")
            nc.vector.tensor_mul(z1, pattn, rkfull)
            rs = small.tile([128, 1], F32, name="rs")
            nc.vector.tensor_reduce(out=rs, in_=z1, axis=AX.X, op=ALU.add)
            rs_s = small.tile([128, 1], F32, name="rs_s")
            nc.vector.tensor_mul(rs_s, rs, rqs)
            sq2 = scratch.tile([128, 128], F32, name="sq2", tag="sq")
            rss = small.tile([128, 1], F32, name="rss")
            nc.scalar.activation(out=sq2, in_=z1, func=ACTF.Square, accum_out=rss)
            rss_s = small.tile([128, 1], F32, name="rss_s")
            nc.vector.tensor_mul(rss_s, rss, rqs2)
            st2 = small.tile([128, 2], F32, name="st2")
            nc.vector.tensor_copy(out=st2[:, 0:1], in_=rs_s)
            nc.vector.tensor_copy(out=st2[:, 1:2], in_=rss_s)
            psst = ps.tile([1, 2], F32, name="psst", tag="ps")
            nc.tensor.matmul(psst, ones, st2)
            mu = small.tile([1, 1], F32, name="mu")
            nc.scalar.mul(mu, psst[0:1, 0:1], 1.0 / (C * C))
            ms = small.tile([1, 1], F32, name="ms")
            nc.scalar.mul(ms, psst[0:1, 1:2], 1.0 / (C * C))
            mu2 = small.tile([1, 1], F32, name="mu2")
            nc.scalar.square(mu2, mu)
            var = small.tile([1, 1], F32, name="var")
            nc.vector.tensor_sub(var, ms, mu2)
            std = small.tile([1, 1], F32, name="std")
            nc.scalar.activation(out=std, in_=var, func=ACTF.Sqrt, bias=epsin)
            rstd1 = small.tile([1, 1], F32, name="rstd1")
            nc.vector.reciprocal(rstd1, std)
            rstdf = small.tile([128, 1], F32, name="rstdf")
            nc.gpsimd.partition_broadcast(rstdf, rstd1)
            # softmax over e of z1*scale_c (instance-norm mean shift cancels)
            scale_c = small.tile([128, 1], F32, name="scale_c")
            nc.vector.tensor_mul(scale_c, rqs, rstdf)
            rm = small.tile([128, 1], F32, name="rm")
            nc.vector.tensor_reduce(out=rm, in_=z1, axis=AX.X, op=ALU.max)
            be = small.tile([128, 1], F32, name="be")
            nc.vector.tensor_mul(be, scale_c, rm)
            be_n = small.tile([128, 1], F32, name="be_n")
            nc.scalar.mul(be_n, be, -1.0)
            expb = med.tile([128, 128], F32, name="expb", tag="expb")
            se = small.tile([128, 1], F32, name="se")
            nc.scalar.activation(out=expb, in_=z1, func=ACTF.Exp,
                                 scale=scale_c, bias=be_n, accum_out=se)
            rse = small.tile([128, 1], F32, name="rse")
            nc.vector.reciprocal(rse, se)
            p_sb = med.tile([128, 128], F32, name="p_sb", tag="p_sb")
            nc.scalar.mul(p_sb, expb, rse)
            st["p_sb"] = p_sb

        def finish_b(st):
            i, p_sb = st["i"], st["p_sb"]
            # P2T = p.T @ Wpo_i.T  -> lhsT for the output matmul
            psp2 = ps.tile([128, 128], F32, name="psp2", tag="ps")
            nc.tensor.matmul(psp2, p_sb, wpo_sb[:, i, :])
            p2t = med.tile([128, 128], BF16, name="p2t", tag="p2t")
            nc.vector.tensor_copy(out=p2t, in_=psp2)
            st["p2t"] = p2t

        def finish_c(st, j0, j1):
            i, p2t, vv = st["i"], st["p2t"], st["vv"]
            for j in range(j0, j1):
                pso = pp.tile([128, 512], F32, name="pso", tag="pp")
                nc.tensor.matmul(pso, p2t, vv[:, j * 512:(j + 1) * 512])
                oc = outp.tile([128, 512], F32, name="oc", tag="oc")
                if j % 2 == 0:
                    nc.vector.tensor_copy(out=oc, in_=pso)
                else:
                    nc.scalar.copy(out=oc, in_=pso)
                nc.sync.dma_start(out=outs_d[i][:, j * 512:(j + 1) * 512], in_=oc)

        prev = [None]

        def stage_hook(s):
            st = prev[0]
            if st is None:
                return
            if s == 2:
                finish_a(st)
            elif s == 3:
                finish_b(st)
            elif s >= 4:
                j0 = st.get("j", 0)
                j1 = min(j0 + 3, 32)
                if j0 < j1:
                    finish_c(st, j0, j1)
                    st["j"] = j1

        def q_slab_hook(s):
            if s < 15:
                q_chunk(2 * s + 2)
                q_chunk(2 * s + 3)
            else:
                q_finalize()

        q_chunk(0)
        q_chunk(1)
        pre_state = {0: prefetch_branch(0)}
        nc.sync.dma_start(out=wpo_sb, in_=wpoT_d.rearrange("i c o -> c i o"))

        def make_pre_hook(nxt):
            def hook():
                pre_state[nxt] = prefetch_branch(nxt)
            return hook

        for i in range(3):
            state = conv_phase(i, pre_state.pop(i),
                               slab_hook=(q_slab_hook if i == 0 else None),
                               stage_hook=stage_hook,
                               pre_hook=make_pre_hook(i + 1) if i < 3 else None)
            prev[0] = state
        # branch 3: k-pass (with attn) first, so the softmax chain and the
        # output matmuls overlap the v-pass instead of trailing the kernel.
        st3 = conv_phase(3, pre_state.pop(3), stage_hook=stage_hook,
                         do_v=False)
        prev[0] = None
        finish_a(st3)
        finish_b(st3)

        def pso_hook(s):
            j0 = st3.get("j", 0)
            j1 = min(max(2 * s - 2, 0), 32)
            if j0 < j1:
                finish_c(st3, j0, j1)
                st3["j"] = j1

        conv_phase(3, None, stage_hook=pso_hook, do_k=False, st=st3)
        finish_c(st3, st3.get("j", 0), 32)


def _build_nc(has_bias):
    kp = 5 if has_bias else 4
    nc = bacc.Bacc("TRN2", target_bir_lowering=False, debug=False, num_devices=8)
    textT_d = nc.dram_tensor("textT", [kp * 128, C], BF16, kind="ExternalInput")
    wqT_d = nc.dram_tensor("wqT", [kp * 128, HW], BF16, kind="ExternalInput")
    embs_d = [nc.dram_tensor(f"emb{i}", [C, HW], BF16, kind="ExternalInput")
              for i in range(4)]
    embs8_d = [nc.dram_tensor(f"emb8_{i}", [C, HW], FP8, kind="ExternalInput")
               for i in range(4)]
    weffk8_d = nc.dram_tensor("weffk8", [4, 9, C, C], FP8, kind="ExternalInput")
    weffv_d = nc.dram_tensor("weffv", [4, 9, C, C], BF16, kind="ExternalInput")
    wpoT_d = nc.dram_tensor("wpoT", [4, C, C], F32, kind="ExternalInput")
    outs_d = [nc.dram_tensor(f"out{i}", [C, HW], F32, kind="ExternalOutput")
              for i in range(4)]
    with tile.TileContext(nc) as tc:
        _body(nc, tc, kp, textT_d, wqT_d, embs_d, embs8_d, weffk8_d, weffv_d,
              wpoT_d, outs_d)
    nc.compile()
    return nc


_NC = {}
_HAS_BIAS = False


def _get_nc():
    if _HAS_BIAS not in _NC:
        _NC[_HAS_BIAS] = _build_nc(_HAS_BIAS)
    return _NC[_HAS_BIAS]


def _prep_in_maps(emb1, emb2, emb3, emb4, text_emb, Wq, bq, Wmk, Wk, Wmv, Wv, Wpo):
    global _HAS_BIAS
    f32 = np.float32
    embs = [np.ascontiguousarray(np.asarray(e, f32).reshape(B, C, HW))
            for e in (emb1, emb2, emb3, emb4)]
    text_emb = np.asarray(text_emb, f32)
    Wq = np.asarray(Wq, f32)
    bq = np.asarray(bq, f32)
    Wmk = np.asarray(Wmk, f32)
    Wk = np.asarray(Wk, f32)
    Wmv = np.asarray(Wmv, f32)
    Wv = np.asarray(Wv, f32)
    Wpo = np.asarray(Wpo, f32)

    _HAS_BIAS = bool(np.any(bq != 0.0))
    kp = 5 if _HAS_BIAS else 4

    wqT = np.zeros((kp * 128, HW), f32)
    wqT[:TS] = Wq.T
    if _HAS_BIAS:
        wqT[TS] = bq
    wqT = wqT.astype(BF16_NP)

    g2 = (np.arange(C) // 2) * 2

    def build_weff(Wm, Wg):
        out = np.empty((4, 9, C, C), f32)
        for i in range(4):
            A = Wg[i][:, 0].reshape(C, 9)
            Bt = Wg[i][:, 1].reshape(C, 9)
            M0 = Wm[i][g2, :]
            M1 = Wm[i][g2 + 1, :]
            out[i] = (np.einsum('ot,oc->tco', A, M0)
                      + np.einsum('ot,oc->tco', Bt, M1)).astype(f32)
        return out

    weffk = build_weff(Wmk, Wk)
    weffv = np.ascontiguousarray(build_weff(Wmv, Wv).astype(BF16_NP))
    # fp8 k weights: DoubleRow tap order, scaled by a per-branch power of two
    # (the scale cancels in the downstream l2norm)
    weffk8 = np.empty((4, 9, C, C), f32)
    for i in range(4):
        m = float(np.abs(weffk[i]).max())
        s = 2.0 ** math.floor(math.log2(120.0 / m)) if m > 0 else 1.0
        weffk8[i] = weffk[i][DR_ORDER] * s
    weffk8 = np.ascontiguousarray(np.clip(weffk8, -240, 240).astype(E4_NP))
    wpoT = np.ascontiguousarray(np.transpose(Wpo, (0, 2, 1)))

    in_maps = []
    for b in range(B):
        textT = np.zeros((kp * 128, C), f32)
        textT[:TS] = text_emb[b, 0].T
        if _HAS_BIAS:
            textT[TS] = 1.0
        textT = textT.astype(BF16_NP)
        m = {"textT": textT, "wqT": wqT, "weffk8": weffk8, "weffv": weffv,
             "wpoT": wpoT}
        for i in range(4):
            eb = embs[i][b]
            m[f"emb{i}"] = np.ascontiguousarray(eb.astype(BF16_NP))
            m[f"emb8_{i}"] = np.ascontiguousarray(eb.astype(E4_NP))
        in_maps.append(m)
    return in_maps


def _run(in_maps, trace=False):
    nc = _get_nc()
    return bass_utils.run_bass_kernel_spmd(nc, in_maps, core_ids=list(range(8)),
                                           trace=trace)


def kernel(emb1, emb2, emb3, emb4, text_emb, Wq, bq, Wmk, Wk, Wmv, Wv, Wpo):
    in_maps = _prep_in_maps(emb1, emb2, emb3, emb4, text_emb, Wq, bq,
                            Wmk, Wk, Wmv, Wv, Wpo)
    res = _run(in_maps, trace=False)
    outs = []
    for i in range(4):
        o = np.stack([res.results[b][f"out{i}"].reshape(C, H, W)
                      for b in range(B)])
        outs.append(np.ascontiguousarray(o.astype(np.float32)))
    return tuple(outs)


# revision 6
# speedup vs baseline: 1.1758x; 1.0266x over previous
"""Trainium2 Bass kernel for the Dblock-ViT channel-attention module.

Strategy: data-parallel over batch (8 batches -> 8 NeuronCores). Each core:
  q       = l2norm(text_emb[b] @ Wq.T + bq)              [C, HW]
  per branch i in 0..3:
    kk    = fused dense 3x3 conv (1x1 folded into taps)  [C, HW]
    attn  = instancenorm(q_n @ kk_n.T / sqrt(C)); p = softmax rows
    out_i = conv(emb_i, What_i),  What_t = weffv_t @ (Wpo_i @ p).T

The k-path conv runs in fp8-e4m3 with DoubleRow perf mode (2 taps per
matmul): 4 DR matmuls + 1 single per 512-wide chunk instead of 9 bf16
matmuls. The k conv output feeds l2norm, so the fp8 weight scaling
cancels and its quantization noise is strongly attenuated by the
softmax. The fp8 slab is zero-padded to 130 columns so every tap runs
full-width with no edge special cases.

The v-path keeps bf16 (its error passes linearly to the output), but the
attention application (Wpo @ p) is folded into the conv weights on
device, so the v conv directly produces the final output: no vv
materialization and no separate output matmuls. Pass order
k0 k1 v0 k2 v1 k3 v2 v3 keeps the PE busy while each branch's softmax
chain and weight fold run during the following pass.
"""

import math
import sys
import types

import ml_dtypes
import numpy as np

BF16_NP = ml_dtypes.bfloat16
E4_NP = ml_dtypes.float8_e4m3

for _p in ("/opt/trn_rl_repo",):
    if _p not in sys.path:
        sys.path.insert(0, _p)

# The image's antenv package lacks axon_hooks; register a functional stand-in
# so run_bass_kernel_spmd(trace=True) can reach the NTFF profiling hook.
try:
    import antenv
    if "antenv.axon_hooks" not in sys.modules:
        _m = types.ModuleType("antenv.axon_hooks")
        _m._hook_val = None
        _m.set_axon_ntff_profile_hook = lambda h: setattr(_m, "_hook_val", h)
        _m.get_axon_ntff_profile_hook = lambda: _m._hook_val
        sys.modules["antenv.axon_hooks"] = _m
        antenv.axon_hooks = _m
        try:
            from trn_agent_boot.trn_boot import _ntff_profile_via_ctypes
            _m._hook_val = _ntff_profile_via_ctypes("/opt/axon/libaxon_pjrt.so")
        except Exception:
            pass
except Exception:
    pass

import concourse.bass as bass
import concourse.mybir as mybir
import concourse.tile as tile
from concourse import bacc, bass_utils
from concourse.ap import AP as BassAP
from concourse.masks import make_identity

try:
    bass_utils.upload_artifacts = lambda tmpdir: tmpdir
except Exception:
    pass

B, C, H, W = 8, 128, 128, 128
HW = H * W
TS = 512
EPS_NORM = 1e-12
EPS_IN = 1e-5
RSQRT_C = 1.0 / math.sqrt(C)
F32 = mybir.dt.float32
BF16 = mybir.dt.bfloat16
FP8 = mybir.dt.float8e4
DRM = mybir.MatmulPerfMode.DoubleRow
TAPS = [(dy, dx) for dy in range(3) for dx in range(3)]
# DoubleRow tap order: 3 dx-pairs (stride 2), 1 dy-pair (stride 260), 1 single
DR_ORDER = [0, 2, 3, 5, 6, 8, 1, 7, 4]
KSL = 1300  # fp8 slab partition stride: 10 rows x 130 cols
AX = mybir.AxisListType
ALU = mybir.AluOpType
ACTF = mybir.ActivationFunctionType


def _body(nc, tc, kp, textT_d, wqT_d, embs_d, embs8_d, weffk8_d, weffvT_d,
          wpoT_d, outs_d):
    from contextlib import ExitStack
    ctx = ExitStack()
    with ctx:
        singles = ctx.enter_context(tc.tile_pool(name="singles", bufs=1))
        small = ctx.enter_context(tc.tile_pool(name="small", bufs=1))
        med = ctx.enter_context(tc.tile_pool(name="med", bufs=2))
        scratch = ctx.enter_context(tc.tile_pool(name="scratch", bufs=2))
        stgp = ctx.enter_context(tc.tile_pool(name="stgp", bufs=3))
        outp = ctx.enter_context(tc.tile_pool(name="outp", bufs=4))
        weffp = ctx.enter_context(tc.tile_pool(name="weffp", bufs=2))
        whatp = ctx.enter_context(tc.tile_pool(name="whatp", bufs=2))
        slabp = ctx.enter_context(tc.tile_pool(name="slabp", bufs=4))
        slab8p = ctx.enter_context(tc.tile_pool(name="slab8p", bufs=4))
        kvp = ctx.enter_context(tc.tile_pool(name="kvp", bufs=2))
        pp = ctx.enter_context(tc.tile_pool(name="pp", bufs=3, space="PSUM"))
        pt = ctx.enter_context(tc.tile_pool(name="pt", bufs=2, space="PSUM"))
        pa = ctx.enter_context(tc.tile_pool(name="pa", bufs=2, space="PSUM"))
        ps = ctx.enter_context(tc.tile_pool(name="ps", bufs=1, space="PSUM"))

        ident_f = singles.tile([128, 128], F32, name="ident_f")
        make_identity(nc, ident_f)
        ident_b = singles.tile([128, 128], BF16, name="ident_b")
        make_identity(nc, ident_b)
        ones = singles.tile([128, 1], F32, name="ones")
        nc.vector.memset(ones, 1.0)
        rkmat = singles.tile([128, 128], F32, name="rkmat")
        nc.vector.memset(rkmat, 0.0)
        epsin = singles.tile([1, 1], F32, name="epsin")
        nc.vector.memset(epsin, EPS_IN)

        wpo_sb = singles.tile([128, 4, 128], F32, name="wpo_sb")

        qT = singles.tile([128, HW], BF16, name="qT")
        qss = singles.tile([128, 32], F32, name="qss")
        rqs = singles.tile([128, 1], F32, name="rqs")
        rqs2 = singles.tile([128, 1], F32, name="rqs2")

        # fp8 slab buffers: zero the padding columns once; the loop DMAs only
        # write cols 1..128 so the zero columns persist across buffer reuse.
        for _zi in range(4):
            zt = slab8p.tile([128, 10, 130], FP8, name=f"kz{_zi}", tag="kslab")
            nc.vector.memset(zt[:, :, 0:1], 0.0)
            nc.vector.memset(zt[:, :, 129:130], 0.0)

        # ---------------- Q phase (emitted interleaved with conv(0)) ----------
        qpool = ctx.enter_context(tc.tile_pool(name="qpool", bufs=1))
        wqp = ctx.enter_context(tc.tile_pool(name="wqp", bufs=3))
        textT_sb = qpool.tile([128, kp, 128], BF16, name="textT_sb")
        nc.sync.dma_start(out=textT_sb,
                          in_=textT_d.rearrange("(k p) c -> p k c", p=128))
        wq_r = wqT_d.rearrange("(k p) n -> p k n", p=128)

        def q_chunk(j):
            wq_t = wqp.tile([128, kp, 512], BF16, name="wq_t", tag="wq_t")
            nc.sync.dma_start(out=wq_t, in_=wq_r[:, :, j * 512:(j + 1) * 512])
            psq = pp.tile([128, 512], F32, name="psq", tag="pp")
            for k in range(kp):
                nc.tensor.matmul(psq,
                                 textT_sb[:, k, :],
                                 wq_t[:, k, :],
                                 start=(k == 0), stop=(k == kp - 1))
            stgq = stgp.tile([128, 512], BF16, name="stgq", tag="stg")
            nc.vector.tensor_copy(out=stgq, in_=psq)
            sqd = scratch.tile([128, 512], BF16, name="sqd", tag="sqd")
            nc.scalar.activation(out=sqd, in_=psq, func=ACTF.Square,
                                 accum_out=qss[:, j:j + 1])
            for b4 in range(4):
                blk = 4 * j + b4
                ptt = pt.tile([128, 128], BF16, name="ptt", tag="pt")
                nc.tensor.transpose(ptt, stgq[:, b4 * 128:(b4 + 1) * 128],
                                    ident_b)
                dst = qT[:, blk * 128:(blk + 1) * 128]
                if blk % 2 == 0:
                    nc.vector.tensor_copy(out=dst, in_=ptt)
                else:
                    nc.scalar.copy(out=dst, in_=ptt)

        def q_finalize():
            qn = small.tile([128, 1], F32, name="qn")
            nc.vector.tensor_reduce(out=qn, in_=qss, axis=AX.X, op=ALU.add)
            nc.scalar.sqrt(qn, qn)
            nc.vector.tensor_scalar_max(qn, qn, EPS_NORM)
            rq = small.tile([128, 1], F32, name="rq")
            nc.vector.reciprocal(rq, qn)
            nc.scalar.mul(rqs, rq, RSQRT_C)
            nc.vector.tensor_mul(rqs2, rqs, rqs)

        # ---------------- slab loading ----------------
        def load_kslab(i, s):
            kslab = slab8p.tile([128, 10, 130], FP8, name=f"ks{i}_{s}",
                                tag="kslab")
            emb8_r = embs8_d[i].rearrange("c (h w) -> c h w", w=128)
            if s == 0:
                nc.vector.memset(kslab[:, 0:1, :], 0.0)
                nc.sync.dma_start(out=kslab[:, 1:10, 1:129],
                                  in_=emb8_r[:, 0:9, :])
            elif s == 15:
                nc.vector.memset(kslab[:, 9:10, :], 0.0)
                nc.sync.dma_start(out=kslab[:, 0:9, 1:129],
                                  in_=emb8_r[:, 119:128, :])
            else:
                nc.sync.dma_start(out=kslab[:, 0:10, 1:129],
                                  in_=emb8_r[:, 8 * s - 1:8 * s + 9, :])
            return kslab

        def load_vslab(i, s, tag="slab"):
            slab = slabp.tile([128, 10, 128], BF16, name=f"vs{i}_{s}", tag=tag)
            emb_r = embs_d[i].rearrange("c (h w) -> c h w", w=128)
            a = max(8 * s - 1, 0)
            b_ = min(8 * s + 9, 128)
            nc.sync.dma_start(out=slab[:, 0:(b_ - a), :], in_=emb_r[:, a:b_, :])
            return slab

        def kconv_chunk(psk, wk_sb, kslab, h2):
            # 4 DoubleRow matmuls (2 taps each) + 1 single; all full 512-wide.
            def rhs(off, slot=None):
                dims = [(KSL, 128)]
                if slot is not None:
                    dims.append((slot, 2))
                dims += [(130, 4), (1, 128)]
                return BassAP(kslab.tensor, kslab.offset + off, dims)
            base = 520 * h2
            nc.tensor.matmul(psk, wk_sb[:, 0:2, :], rhs(base, 2),
                             start=True, stop=False, perf_mode=DRM,
                             skip_group_check=True)
            nc.tensor.matmul(psk, wk_sb[:, 2:4, :], rhs(base + 130, 2),
                             start=False, stop=False, perf_mode=DRM,
                             skip_group_check=True)
            nc.tensor.matmul(psk, wk_sb[:, 4:6, :], rhs(base + 260, 2),
                             start=False, stop=False, perf_mode=DRM,
                             skip_group_check=True)
            nc.tensor.matmul(psk, wk_sb[:, 6:8, :], rhs(base + 1, 260),
                             start=False, stop=False, perf_mode=DRM,
                             skip_group_check=True)
            nc.tensor.matmul(psk, wk_sb[:, 8, :], rhs(base + 131),
                             start=False, stop=True, skip_group_check=True)

        def vconv_chunk(psum, w_sb, slab, s, h2):
            # center tap (dy=1,dx=1) first: it always covers the full
            # chunk, so start=True initializes every psum element.
            order = [4, 0, 1, 2, 3, 5, 6, 7, 8]
            for n_t, t in enumerate(order):
                dy, dx = TAPS[t]
                rr0 = 1 if (s == 0 and h2 == 0 and dy == 0) else 0
                rr1 = 3 if (s == 15 and h2 == 1 and dy == 2) else 4
                base = 4 * h2 + dy - (1 if s == 0 else 0)
                co0, co1 = (1, 128) if dx == 0 else (0, 127) if dx == 2 else (0, 128)
                ci0, ci1 = (0, 127) if dx == 0 else (1, 128) if dx == 2 else (0, 128)
                nc.tensor.matmul(psum[:, rr0:rr1, co0:co1],
                                 w_sb[:, t, :],
                                 slab[:, base + rr0:base + rr1, ci0:ci1],
                                 start=(n_t == 0), stop=(n_t == 8),
                                 skip_group_check=True)

        # ---------------- prefetch ----------------
        def prefetch_k(i):
            wk_sb = weffp.tile([128, 9, 128], FP8, name=f"wk{i}", tag="wk")
            nc.sync.dma_start(out=wk_sb,
                              in_=weffk8_d[i].rearrange("t c o -> c t o"))
            # wvT is needed by the weight fold right after the k pass ends
            wvT_sb = weffp.tile([128, 9, 128], BF16, name=f"wvT{i}", tag="wv")
            nc.sync.dma_start(out=wvT_sb,
                              in_=weffvT_d[i].rearrange("t e c -> e t c"))
            return {"wk_sb": wk_sb, "wvT_sb": wvT_sb, "kslab0": load_kslab(i, 0)}

        def prefetch_v(i):
            return {"vslab0": load_vslab(i, 0)}

        # ---------------- passes ----------------
        def kpass(i, st, slab_hook=None, stage_hook=None, pre_hook=None):
            wk_sb = st["wk_sb"]
            pattn = pa.tile([128, 128], F32, name=f"pattn{i}", tag="pa")
            kssb = kvp.tile([128, 32], F32, name=f"kss{i}", tag="kss")
            st["pattn"], st["kssb"] = pattn, kssb
            for s in range(16):
                if slab_hook is not None:
                    slab_hook(s)
                if stage_hook is not None:
                    stage_hook(s)
                if s == 13 and pre_hook is not None:
                    pre_hook()
                kslab = st.pop("kslab0") if s == 0 else load_kslab(i, s)
                for h2 in range(2):
                    j = 2 * s + h2
                    psk = pp.tile([128, 512], F32, name="psk", tag="pp")
                    kconv_chunk(psk, wk_sb, kslab, h2)
                    stgk = stgp.tile([128, 512], BF16, name="stgk", tag="stg")
                    nc.vector.tensor_copy(out=stgk, in_=psk)
                    sqd = scratch.tile([128, 512], BF16, name="sqd", tag="sqd")
                    nc.scalar.activation(out=sqd, in_=psk, func=ACTF.Square,
                                         accum_out=kssb[:, j:j + 1])
                    # transpose each 128-block and accumulate attn inline
                    for b4 in range(4):
                        jj = 4 * j + b4
                        ptt = pt.tile([128, 128], BF16, name="ptk", tag="pt")
                        nc.tensor.transpose(ptt, stgk[:, b4 * 128:(b4 + 1) * 128],
                                            ident_b)
                        ktb = stgp.tile([128, 128], BF16, name="ktb", tag="ktb")
                        if jj % 2 == 0:
                            nc.vector.tensor_copy(out=ktb, in_=ptt)
                        else:
                            nc.scalar.copy(out=ktb, in_=ptt)
                        nc.tensor.matmul(pattn,
                                         qT[:, jj * 128:(jj + 1) * 128], ktb,
                                         start=(jj == 0), stop=(jj == 127),
                                         skip_group_check=True)
            return st

        def vpass(i, st, stage_hook=None, pre_hook=None):
            what = st["what"]
            for s in range(16):
                if stage_hook is not None:
                    stage_hook(s)
                if s == 13 and pre_hook is not None:
                    pre_hook()
                slab = st.pop("vslab0") if s == 0 else load_vslab(i, s)
                for h2 in range(2):
                    j = 2 * s + h2
                    psv = pp.tile([128, 4, 128], F32, name="psv", tag="pp")
                    vconv_chunk(psv, what, slab, s, h2)
                    oc = outp.tile([128, 512], F32, name="oc", tag="oc")
                    if j % 2 == 0:
                        nc.vector.tensor_copy(
                            out=oc, in_=psv.rearrange("p a b -> p (a b)"))
                    else:
                        nc.scalar.copy(
                            out=oc, in_=psv.rearrange("p a b -> p (a b)"))
                    nc.sync.dma_start(out=outs_d[i][:, j * 512:(j + 1) * 512],
                                      in_=oc)
            return st

        # ---------------- per-branch finish chain ----------------
        def finish_a(st):
            pattn, kssb = st["pattn"], st["kssb"]
            # kk row norms -> rk, transposed into a broadcast row
            kn = small.tile([128, 1], F32, name="kn")
            nc.vector.tensor_reduce(out=kn, in_=kssb, axis=AX.X, op=ALU.add)
            nc.scalar.sqrt(kn, kn)
            nc.vector.tensor_scalar_max(kn, kn, EPS_NORM)
            rk = small.tile([128, 1], F32, name="rk")
            nc.vector.reciprocal(rk, kn)
            nc.vector.tensor_copy(out=rkmat[:, 0:1], in_=rk)
            psrk = ps.tile([128, 128], F32, name="psrk", tag="ps")
            nc.tensor.transpose(psrk, rkmat, ident_f)
            rkrow = small.tile([1, 128], F32, name="rkrow")
            nc.vector.tensor_copy(out=rkrow, in_=psrk[0:1, :])
            rkfull = med.tile([128, 128], F32, name="rkfull", tag="rkfull")
            nc.gpsimd.partition_broadcast(rkfull, rkrow)

            # z1 = attn_raw * rk[e];  stats of attn_s = z1 * rqs[c]
            z1 = med.tile([128, 128], F32, name="z1", tag="z1")
            nc.vector.tensor_mul(z1, pattn, rkfull)
            rs = small.tile([128, 1], F32, name="rs")
            nc.vector.tensor_reduce(out=rs, in_=z1, axis=AX.X, op=ALU.add)
            rs_s = small.tile([128, 1], F32, name="rs_s")
            nc.vector.tensor_mul(rs_s, rs, rqs)
            sq2 = scratch.tile([128, 128], F32, name="sq2", tag="sq")
            rss = small.tile([128, 1], F32, name="rss")
            nc.scalar.activation(out=sq2, in_=z1, func=ACTF.Square, accum_out=rss)
            rss_s = small.tile([128, 1], F32, name="rss_s")
            nc.vector.tensor_mul(rss_s, rss, rqs2)
            st2 = small.tile([128, 2], F32, name="st2")
            nc.vector.tensor_copy(out=st2[:, 0:1], in_=rs_s)
            nc.vector.tensor_copy(out=st2[:, 1:2], in_=rss_s)
            psst = ps.tile([1, 2], F32, name="psst", tag="ps")
            nc.tensor.matmul(psst, ones, st2)
            mu = small.tile([1, 1], F32, name="mu")
            nc.scalar.mul(mu, psst[0:1, 0:1], 1.0 / (C * C))
            ms = small.tile([1, 1], F32, name="ms")
            nc.scalar.mul(ms, psst[0:1, 1:2], 1.0 / (C * C))
            mu2 = small.tile([1, 1], F32, name="mu2")
            nc.scalar.square(mu2, mu)
            var = small.tile([1, 1], F32, name="var")
            nc.vector.tensor_sub(var, ms, mu2)
            std = small.tile([1, 1], F32, name="std")
            nc.scalar.activation(out=std, in_=var, func=ACTF.Sqrt, bias=epsin)
            rstd1 = small.tile([1, 1], F32, name="rstd1")
            nc.vector.reciprocal(rstd1, std)
            rstdf = small.tile([128, 1], F32, name="rstdf")
            nc.gpsimd.partition_broadcast(rstdf, rstd1)
            # softmax over e of z1*scale_c (instance-norm mean shift cancels)
            scale_c = small.tile([128, 1], F32, name="scale_c")
            nc.vector.tensor_mul(scale_c, rqs, rstdf)
            rm = small.tile([128, 1], F32, name="rm")
            nc.vector.tensor_reduce(out=rm, in_=z1, axis=AX.X, op=ALU.max)
            be = small.tile([128, 1], F32, name="be")
            nc.vector.tensor_mul(be, scale_c, rm)
            be_n = small.tile([128, 1], F32, name="be_n")
            nc.scalar.mul(be_n, be, -1.0)
            expb = med.tile([128, 128], F32, name="expb", tag="expb")
            se = small.tile([128, 1], F32, name="se")
            nc.scalar.activation(out=expb, in_=z1, func=ACTF.Exp,
                                 scale=scale_c, bias=be_n, accum_out=se)
            rse = small.tile([128, 1], F32, name="rse")
            nc.vector.reciprocal(rse, se)
            p_sb = med.tile([128, 128], F32, name="p_sb", tag="p_sb")
            nc.scalar.mul(p_sb, expb, rse)
            st["p_sb"] = p_sb

        def finish_b(st):
            i, p_sb = st["i"], st["p_sb"]
            # P2T = p.T @ Wpo_i.T  -> [e, o] operand for the weight fold
            psp2 = ps.tile([128, 128], F32, name="psp2", tag="ps")
            nc.tensor.matmul(psp2, p_sb, wpo_sb[:, i, :])
            p2t = med.tile([128, 128], BF16, name="p2t", tag="p2t")
            nc.vector.tensor_copy(out=p2t, in_=psp2)
            st["p2t"] = p2t

        def finish_w(st, batch):
            # What_t[c,o] = sum_e weffv_t[c,e] * (Wpo @ p)[o,e]; 3 taps/batch
            p2t, wvT_sb = st["p2t"], st["wvT_sb"]
            if batch == 0:
                st["what"] = whatp.tile([128, 9, 128], BF16, name="what",
                                        tag="what")
            what = st["what"]
            psw = ps.tile([128, 3, 128], F32, name="psw", tag="ps")
            for tt in range(3):
                t = 3 * batch + tt
                nc.tensor.matmul(psw[:, tt, :], wvT_sb[:, t, :], p2t,
                                 skip_group_check=True)
            nc.vector.tensor_copy(out=what[:, 3 * batch:3 * batch + 3, :],
                                  in_=psw)

        def make_fin_hook(st):
            def hook(s):
                if s == 2:
                    finish_a(st)
                elif s == 3:
                    finish_b(st)
                elif s in (4, 5, 6):
                    finish_w(st, s - 4)
            return hook

        def q_slab_hook(s):
            if s < 15:
                q_chunk(2 * s + 2)
                q_chunk(2 * s + 3)
            else:
                q_finalize()

        # ---------------- main schedule ----------------
        q_chunk(0)
        q_chunk(1)
        sts = {i: {"i": i} for i in range(4)}
        sts[0].update(prefetch_k(0))
        nc.sync.dma_start(out=wpo_sb, in_=wpoT_d.rearrange("i c o -> c i o"))

        # pass sequence: k0 k1 v0 k2 v1 k3 v2 v3
        seq = [("k", 0), ("k", 1), ("v", 0), ("k", 2), ("v", 1), ("k", 3),
               ("v", 2), ("v", 3)]

        def make_pre_hook(n):
            if n + 1 >= len(seq):
                return None
            kind, i = seq[n + 1]

            def hook():
                sts[i].update(prefetch_k(i) if kind == "k" else prefetch_v(i))
            return hook

        fin_i = [None]
        for n, (kind, i) in enumerate(seq):
            fin = make_fin_hook(sts[fin_i[0]]) if fin_i[0] is not None else None
            if kind == "k":
                kpass(i, sts[i], slab_hook=(q_slab_hook if n == 0 else None),
                      stage_hook=fin, pre_hook=make_pre_hook(n))
                fin_i[0] = i
            else:
                vpass(i, sts[i], stage_hook=fin, pre_hook=make_pre_hook(n))
                fin_i[0] = None


def _build_nc(has_bias):
    kp = 5 if has_bias else 4
    nc = bacc.Bacc("TRN2", target_bir_lowering=False, debug=False, num_devices=8)
    textT_d = nc.dram_tensor("textT", [kp * 128, C], BF16, kind="ExternalInput")
    wqT_d = nc.dram_tensor("wqT", [kp * 128, HW], BF16, kind="ExternalInput")
    embs_d = [nc.dram_tensor(f"emb{i}", [C, HW], BF16, kind="ExternalInput")
              for i in range(4)]
    embs8_d = [nc.dram_tensor(f"emb8_{i}", [C, HW], FP8, kind="ExternalInput")
               for i in range(4)]
    weffk8_d = nc.dram_tensor("weffk8", [4, 9, C, C], FP8, kind="ExternalInput")
    weffvT_d = nc.dram_tensor("weffvT", [4, 9, C, C], BF16, kind="ExternalInput")
    wpoT_d = nc.dram_tensor("wpoT", [4, C, C], F32, kind="ExternalInput")
    outs_d = [nc.dram_tensor(f"out{i}", [C, HW], F32, kind="ExternalOutput")
              for i in range(4)]
    with tile.TileContext(nc) as tc:
        _body(nc, tc, kp, textT_d, wqT_d, embs_d, embs8_d, weffk8_d, weffvT_d,
              wpoT_d, outs_d)
    nc.compile()
    return nc


_NC = {}
_HAS_BIAS = False


def _get_nc():
    if _HAS_BIAS not in _NC:
        _NC[_HAS_BIAS] = _build_nc(_HAS_BIAS)
    return _NC[_HAS_BIAS]


def _prep_in_maps(emb1, emb2, emb3, emb4, text_emb, Wq, bq, Wmk, Wk, Wmv, Wv, Wpo):
    global _HAS_BIAS
    f32 = np.float32
    embs = [np.ascontiguousarray(np.asarray(e, f32).reshape(B, C, HW))
            for e in (emb1, emb2, emb3, emb4)]
    text_emb = np.asarray(text_emb, f32)
    Wq = np.asarray(Wq, f32)
    bq = np.asarray(bq, f32)
    Wmk = np.asarray(Wmk, f32)
    Wk = np.asarray(Wk, f32)
    Wmv = np.asarray(Wmv, f32)
    Wv = np.asarray(Wv, f32)
    Wpo = np.asarray(Wpo, f32)

    _HAS_BIAS = bool(np.any(bq != 0.0))
    kp = 5 if _HAS_BIAS else 4

    wqT = np.zeros((kp * 128, HW), f32)
    wqT[:TS] = Wq.T
    if _HAS_BIAS:
        wqT[TS] = bq
    wqT = wqT.astype(BF16_NP)

    g2 = (np.arange(C) // 2) * 2

    def build_weff(Wm, Wg):
        out = np.empty((4, 9, C, C), f32)
        for i in range(4):
            A = Wg[i][:, 0].reshape(C, 9)
            Bt = Wg[i][:, 1].reshape(C, 9)
            M0 = Wm[i][g2, :]
            M1 = Wm[i][g2 + 1, :]
            out[i] = (np.einsum('ot,oc->tco', A, M0)
                      + np.einsum('ot,oc->tco', Bt, M1)).astype(f32)
        return out

    weffk = build_weff(Wmk, Wk)
    # v weights transposed per tap: [t, e, c] for the on-device weight fold
    weffvT = np.ascontiguousarray(
        np.transpose(build_weff(Wmv, Wv), (0, 1, 3, 2)).astype(BF16_NP))
    # fp8 k weights: DoubleRow tap order, scaled by a per-branch power of two
    # (the scale cancels in the downstream l2norm)
    weffk8 = np.empty((4, 9, C, C), f32)
    for i in range(4):
        m = float(np.abs(weffk[i]).max())
        s = 2.0 ** math.floor(math.log2(120.0 / m)) if m > 0 else 1.0
        weffk8[i] = weffk[i][DR_ORDER] * s
    weffk8 = np.ascontiguousarray(np.clip(weffk8, -240, 240).astype(E4_NP))
    wpoT = np.ascontiguousarray(np.transpose(Wpo, (0, 2, 1)))

    in_maps = []
    for b in range(B):
        textT = np.zeros((kp * 128, C), f32)
        textT[:TS] = text_emb[b, 0].T
        if _HAS_BIAS:
            textT[TS] = 1.0
        textT = textT.astype(BF16_NP)
        m = {"textT": textT, "wqT": wqT, "weffk8": weffk8, "weffvT": weffvT,
             "wpoT": wpoT}
        for i in range(4):
            eb = embs[i][b]
            m[f"emb{i}"] = np.ascontiguousarray(eb.astype(BF16_NP))
            m[f"emb8_{i}"] = np.ascontiguousarray(eb.astype(E4_NP))
        in_maps.append(m)
    return in_maps


def _run(in_maps, trace=False):
    nc = _get_nc()
    return bass_utils.run_bass_kernel_spmd(nc, in_maps, core_ids=list(range(8)),
                                           trace=trace)


def kernel(emb1, emb2, emb3, emb4, text_emb, Wq, bq, Wmk, Wk, Wmv, Wv, Wpo):
    in_maps = _prep_in_maps(emb1, emb2, emb3, emb4, text_emb, Wq, bq,
                            Wmk, Wk, Wmv, Wv, Wpo)
    res = _run(in_maps, trace=False)
    outs = []
    for i in range(4):
        o = np.stack([res.results[b][f"out{i}"].reshape(C, H, W)
                      for b in range(B)])
        outs.append(np.ascontiguousarray(o.astype(np.float32)))
    return tuple(outs)


# revision 8
# speedup vs baseline: 1.3949x; 1.1863x over previous
"""Trainium2 Bass kernel for the Dblock-ViT channel-attention module.

Strategy: data-parallel over batch (8 batches -> 8 NeuronCores). Each core:
  q       = l2norm(text_emb[b] @ Wq.T + bq)              [C, HW]
  per branch i in 0..3:
    kk    = fused dense 3x3 conv (1x1 folded into taps)  [C, HW]
    attn  = instancenorm(q_n @ kk_n.T / sqrt(C)); p = softmax rows
    out_i = conv(emb_i, What_i),  What_t = weffv_t @ (Wpo_i @ p).T

The k-path conv runs in fp8-e4m3 with DoubleRow perf mode (2 taps per
matmul): 4 DR matmuls + 1 single per 512-wide chunk instead of 9 bf16
matmuls. The k conv output feeds l2norm, so the fp8 weight scaling
cancels and its quantization noise is strongly attenuated by the
softmax. The fp8 slab is zero-padded to 130 columns so every tap runs
full-width with no edge special cases.

The v-path keeps bf16 (its error passes linearly to the output), but the
attention application (Wpo @ p) is folded into the conv weights on
device, so the v conv directly produces the final output: no vv
materialization and no separate output matmuls. Pass order
k0 k1 v0 k2 v1 k3 v2 v3 keeps the PE busy while each branch's softmax
chain and weight fold run during the following pass.
"""

import math
import sys
import types

import ml_dtypes
import numpy as np

BF16_NP = ml_dtypes.bfloat16
E4_NP = ml_dtypes.float8_e4m3

for _p in ("/opt/trn_rl_repo",):
    if _p not in sys.path:
        sys.path.insert(0, _p)

# The image's antenv package lacks axon_hooks; register a functional stand-in
# so run_bass_kernel_spmd(trace=True) can reach the NTFF profiling hook.
try:
    import antenv
    if "antenv.axon_hooks" not in sys.modules:
        _m = types.ModuleType("antenv.axon_hooks")
        _m._hook_val = None
        _m.set_axon_ntff_profile_hook = lambda h: setattr(_m, "_hook_val", h)
        _m.get_axon_ntff_profile_hook = lambda: _m._hook_val
        sys.modules["antenv.axon_hooks"] = _m
        antenv.axon_hooks = _m
        try:
            from trn_agent_boot.trn_boot import _ntff_profile_via_ctypes
            _m._hook_val = _ntff_profile_via_ctypes("/opt/axon/libaxon_pjrt.so")
        except Exception:
            pass
except Exception:
    pass

import concourse.bass as bass
import concourse.mybir as mybir
import concourse.tile as tile
from concourse import bacc, bass_utils
from concourse.ap import AP as BassAP
from concourse.masks import make_identity

try:
    bass_utils.upload_artifacts = lambda tmpdir: tmpdir
except Exception:
    pass

B, C, H, W = 8, 128, 128, 128
HW = H * W
TS = 512
EPS_NORM = 1e-12
EPS_IN = 1e-5
RSQRT_C = 1.0 / math.sqrt(C)
F32 = mybir.dt.float32
BF16 = mybir.dt.bfloat16
FP8 = mybir.dt.float8e4
DRM = mybir.MatmulPerfMode.DoubleRow
TAPS = [(dy, dx) for dy in range(3) for dx in range(3)]
# DoubleRow tap order: 3 dx-pairs (stride 2), 1 dy-pair (stride 260), 1 single
DR_ORDER = [0, 2, 3, 5, 6, 8, 1, 7, 4]
KSL = 1300  # fp8 slab partition stride: 10 rows x 130 cols
AX = mybir.AxisListType
ALU = mybir.AluOpType
ACTF = mybir.ActivationFunctionType


def _body(nc, tc, kp, textT_d, wqT_d, embs_d, embs8_d, weffk8_d, weffvT_d,
          wpoT_d, outs_d):
    from contextlib import ExitStack
    ctx = ExitStack()
    with ctx:
        singles = ctx.enter_context(tc.tile_pool(name="singles", bufs=1))
        small = ctx.enter_context(tc.tile_pool(name="small", bufs=1))
        med = ctx.enter_context(tc.tile_pool(name="med", bufs=2))
        scratch = ctx.enter_context(tc.tile_pool(name="scratch", bufs=2))
        stgp = ctx.enter_context(tc.tile_pool(name="stgp", bufs=3))
        outp = ctx.enter_context(tc.tile_pool(name="outp", bufs=4))
        weffp = ctx.enter_context(tc.tile_pool(name="weffp", bufs=2))
        whatp = ctx.enter_context(tc.tile_pool(name="whatp", bufs=2))
        slabp = ctx.enter_context(tc.tile_pool(name="slabp", bufs=4))
        slab8p = ctx.enter_context(tc.tile_pool(name="slab8p", bufs=4))
        kvp = ctx.enter_context(tc.tile_pool(name="kvp", bufs=2))
        pp = ctx.enter_context(tc.tile_pool(name="pp", bufs=3, space="PSUM"))
        pt = ctx.enter_context(tc.tile_pool(name="pt", bufs=2, space="PSUM"))
        pa = ctx.enter_context(tc.tile_pool(name="pa", bufs=2, space="PSUM"))
        ps = ctx.enter_context(tc.tile_pool(name="ps", bufs=1, space="PSUM"))

        ident_f = singles.tile([128, 128], F32, name="ident_f")
        make_identity(nc, ident_f)
        ident_b = singles.tile([128, 128], BF16, name="ident_b")
        make_identity(nc, ident_b)
        ones = singles.tile([128, 1], F32, name="ones")
        nc.vector.memset(ones, 1.0)
        rkmat = singles.tile([128, 128], F32, name="rkmat")
        nc.vector.memset(rkmat, 0.0)
        epsin = singles.tile([1, 1], F32, name="epsin")
        nc.vector.memset(epsin, EPS_IN)

        wpo_sb = singles.tile([128, 4, 128], F32, name="wpo_sb")

        qT = singles.tile([128, HW], FP8, name="qT")
        qss = singles.tile([128, 32], F32, name="qss")
        rqs = singles.tile([128, 1], F32, name="rqs")
        rqs2 = singles.tile([128, 1], F32, name="rqs2")

        # fp8 slab buffers: zero the padding columns once; the loop DMAs only
        # write cols 1..128 so the zero columns persist across buffer reuse.
        for _zi in range(4):
            zt = slab8p.tile([128, 10, 130], FP8, name=f"kz{_zi}", tag="kslab")
            nc.vector.memset(zt[:, :, 0:1], 0.0)
            nc.vector.memset(zt[:, :, 129:130], 0.0)

        # ---------------- Q phase (emitted interleaved with conv(0)) ----------
        qpool = ctx.enter_context(tc.tile_pool(name="qpool", bufs=1))
        wqp = ctx.enter_context(tc.tile_pool(name="wqp", bufs=4))
        textT_sb = qpool.tile([128, kp, 128], FP8, name="textT_sb")
        nc.sync.dma_start(out=textT_sb,
                          in_=textT_d.rearrange("(k p) c -> p k c", p=128))
        wq_r = wqT_d.rearrange("(k p) n -> p k n", p=128)

        def q_chunk(j):
            wq_t = wqp.tile([128, kp, 512], FP8, name="wq_t", tag="wq_t")
            nc.sync.dma_start(out=wq_t, in_=wq_r[:, :, j * 512:(j + 1) * 512])
            psq = pp.tile([128, 512], F32, name="psq", tag="pp")
            for m in range(kp // 2):
                nc.tensor.matmul(psq,
                                 textT_sb[:, 2 * m:2 * m + 2, :],
                                 wq_t[:, 2 * m:2 * m + 2, :],
                                 start=(m == 0), stop=(kp % 2 == 0 and m == kp // 2 - 1),
                                 perf_mode=DRM, skip_group_check=True)
            if kp % 2 == 1:
                nc.tensor.matmul(psq, textT_sb[:, kp - 1, :],
                                 wq_t[:, kp - 1, :],
                                 start=False, stop=True, skip_group_check=True)
            stgq = stgp.tile([128, 512], BF16, name="stgq", tag="stg")
            nc.vector.tensor_copy(out=stgq, in_=psq)
            sqd = scratch.tile([128, 512], BF16, name="sqd", tag="sqd")
            nc.scalar.activation(out=sqd, in_=psq, func=ACTF.Square,
                                 accum_out=qss[:, j:j + 1])
            ptq = pt.tile([128, 4, 128], BF16, name="ptq", tag="pt")
            for b4 in range(4):
                nc.tensor.matmul(ptq[:, b4, :],
                                 stgq[:, b4 * 128:(b4 + 1) * 128], ident_b,
                                 is_transpose=True, skip_group_check=True)
            dst = qT[:, j * 512:(j + 1) * 512]
            if j % 2 == 0:
                nc.vector.tensor_copy(out=dst,
                                      in_=ptq.rearrange("p a b -> p (a b)"))
            else:
                nc.scalar.copy(out=dst, in_=ptq.rearrange("p a b -> p (a b)"))

        def q_finalize():
            qn = small.tile([128, 1], F32, name="qn")
            nc.vector.tensor_reduce(out=qn, in_=qss, axis=AX.X, op=ALU.add)
            nc.scalar.sqrt(qn, qn)
            nc.vector.tensor_scalar_max(qn, qn, EPS_NORM)
            rq = small.tile([128, 1], F32, name="rq")
            nc.vector.reciprocal(rq, qn)
            nc.scalar.mul(rqs, rq, RSQRT_C)
            nc.vector.tensor_mul(rqs2, rqs, rqs)

        # ---------------- slab loading ----------------
        def load_kslab(i, s):
            kslab = slab8p.tile([128, 10, 130], FP8, name=f"ks{i}_{s}",
                                tag="kslab")
            emb8_r = embs8_d[i].rearrange("c (h w) -> c h w", w=128)
            if s == 0:
                nc.vector.memset(kslab[:, 0:1, :], 0.0)
                nc.sync.dma_start(out=kslab[:, 1:10, 1:129],
                                  in_=emb8_r[:, 0:9, :])
            elif s == 15:
                nc.vector.memset(kslab[:, 9:10, :], 0.0)
                nc.sync.dma_start(out=kslab[:, 0:9, 1:129],
                                  in_=emb8_r[:, 119:128, :])
            else:
                nc.sync.dma_start(out=kslab[:, 0:10, 1:129],
                                  in_=emb8_r[:, 8 * s - 1:8 * s + 9, :])
            return kslab

        def load_vslab(i, s, tag="slab"):
            slab = slabp.tile([128, 10, 128], BF16, name=f"vs{i}_{s}", tag=tag)
            emb_r = embs_d[i].rearrange("c (h w) -> c h w", w=128)
            a = max(8 * s - 1, 0)
            b_ = min(8 * s + 9, 128)
            nc.sync.dma_start(out=slab[:, 0:(b_ - a), :], in_=emb_r[:, a:b_, :])
            return slab

        def kconv_chunk(psk, wk_sb, kslab, h2):
            # 4 DoubleRow matmuls (2 taps each) + 1 single; all full 512-wide.
            def rhs(off, slot=None):
                dims = [(KSL, 128)]
                if slot is not None:
                    dims.append((slot, 2))
                dims += [(130, 4), (1, 128)]
                return BassAP(kslab.tensor, kslab.offset + off, dims)
            base = 520 * h2
            nc.tensor.matmul(psk, wk_sb[:, 0:2, :], rhs(base, 2),
                             start=True, stop=False, perf_mode=DRM,
                             skip_group_check=True)
            nc.tensor.matmul(psk, wk_sb[:, 2:4, :], rhs(base + 130, 2),
                             start=False, stop=False, perf_mode=DRM,
                             skip_group_check=True)
            nc.tensor.matmul(psk, wk_sb[:, 4:6, :], rhs(base + 260, 2),
                             start=False, stop=False, perf_mode=DRM,
                             skip_group_check=True)
            nc.tensor.matmul(psk, wk_sb[:, 6:8, :], rhs(base + 1, 260),
                             start=False, stop=False, perf_mode=DRM,
                             skip_group_check=True)
            nc.tensor.matmul(psk, wk_sb[:, 8, :], rhs(base + 131),
                             start=False, stop=True, skip_group_check=True)

        def vconv_chunk(psum, w_sb, slab, s, h2):
            # center tap (dy=1,dx=1) first: it always covers the full
            # chunk, so start=True initializes every psum element.
            order = [4, 0, 1, 2, 3, 5, 6, 7, 8]
            for n_t, t in enumerate(order):
                dy, dx = TAPS[t]
                rr0 = 1 if (s == 0 and h2 == 0 and dy == 0) else 0
                rr1 = 3 if (s == 15 and h2 == 1 and dy == 2) else 4
                base = 4 * h2 + dy - (1 if s == 0 else 0)
                co0, co1 = (1, 128) if dx == 0 else (0, 127) if dx == 2 else (0, 128)
                ci0, ci1 = (0, 127) if dx == 0 else (1, 128) if dx == 2 else (0, 128)
                nc.tensor.matmul(psum[:, rr0:rr1, co0:co1],
                                 w_sb[:, t, :],
                                 slab[:, base + rr0:base + rr1, ci0:ci1],
                                 start=(n_t == 0), stop=(n_t == 8),
                                 skip_group_check=True)

        # ---------------- prefetch ----------------
        def prefetch_k(i):
            wk_sb = weffp.tile([128, 9, 128], FP8, name=f"wk{i}", tag="wk")
            nc.sync.dma_start(out=wk_sb,
                              in_=weffk8_d[i].rearrange("t c o -> c t o"))
            # wvT is needed by the weight fold right after the k pass ends
            wvT_sb = weffp.tile([128, 9, 128], BF16, name=f"wvT{i}", tag="wv")
            nc.sync.dma_start(out=wvT_sb,
                              in_=weffvT_d[i].rearrange("t e c -> e t c"))
            return {"wk_sb": wk_sb, "wvT_sb": wvT_sb, "kslab0": load_kslab(i, 0)}

        def prefetch_v(i):
            return {"vslab0": load_vslab(i, 0)}

        # ---------------- passes ----------------
        def kpass(i, st, slab_hook=None, stage_hook=None, pre_hook=None):
            wk_sb = st["wk_sb"]
            pattn = pa.tile([128, 128], F32, name=f"pattn{i}", tag="pa")
            kssb = kvp.tile([128, 32], F32, name=f"kss{i}", tag="kss")
            st["pattn"], st["kssb"] = pattn, kssb
            def attn_mms(ktb4, j):
                # 2 DoubleRow matmuls: 4 contraction blocks of 128
                for b in range(2):
                    lhsq = BassAP(qT.tensor,
                                  qT.offset + (4 * j + 2 * b) * 128,
                                  [(HW, 128), (128, 2), (1, 128)])
                    rhsk = BassAP(ktb4.tensor, ktb4.offset + 256 * b,
                                  [(512, 128), (128, 2), (1, 128)])
                    nc.tensor.matmul(pattn, lhsq, rhsk,
                                     start=(j == 0 and b == 0),
                                     stop=(j == 31 and b == 1),
                                     perf_mode=DRM, skip_group_check=True)

            pend = []
            for s in range(16):
                if slab_hook is not None:
                    slab_hook(s)
                if stage_hook is not None:
                    stage_hook(s)
                if s == 13 and pre_hook is not None:
                    pre_hook()
                kslab = st.pop("kslab0") if s == 0 else load_kslab(i, s)
                for h2 in range(2):
                    j = 2 * s + h2
                    psk = pp.tile([128, 512], F32, name="psk", tag="pp")
                    kconv_chunk(psk, wk_sb, kslab, h2)
                    stgk = stgp.tile([128, 512], BF16, name="stgk", tag="stg")
                    nc.vector.tensor_copy(out=stgk, in_=psk)
                    sqd = scratch.tile([128, 512], BF16, name="sqd", tag="sqd")
                    nc.scalar.activation(out=sqd, in_=psk, func=ACTF.Square,
                                         accum_out=kssb[:, j:j + 1])
                    ptk = pt.tile([128, 4, 128], BF16, name="ptk", tag="pt")
                    for b4 in range(4):
                        nc.tensor.matmul(ptk[:, b4, :],
                                         stgk[:, b4 * 128:(b4 + 1) * 128],
                                         ident_b, is_transpose=True,
                                         skip_group_check=True)
                    ktb4 = stgp.tile([128, 512], FP8, name="ktb4", tag="ktb")
                    if j % 2 == 0:
                        nc.vector.tensor_copy(
                            out=ktb4, in_=ptk.rearrange("p a b -> p (a b)"))
                    else:
                        nc.scalar.copy(
                            out=ktb4, in_=ptk.rearrange("p a b -> p (a b)"))
                    # attn matmuls run one chunk behind so the ktb4 copy is
                    # never on the PE critical path
                    pend.append((ktb4, j))
                    if len(pend) > 1:
                        attn_mms(*pend.pop(0))
            attn_mms(*pend.pop(0))
            return st

        def vpass(i, st, stage_hook=None, pre_hook=None):
            what = st["what"]
            for s in range(16):
                if stage_hook is not None:
                    stage_hook(s)
                if s == 13 and pre_hook is not None:
                    pre_hook()
                slab = st.pop("vslab0") if s == 0 else load_vslab(i, s)
                for h2 in range(2):
                    j = 2 * s + h2
                    psv = pp.tile([128, 4, 128], F32, name="psv", tag="pp")
                    vconv_chunk(psv, what, slab, s, h2)
                    oc = outp.tile([128, 512], F32, name="oc", tag="oc")
                    if j % 2 == 0:
                        nc.vector.tensor_copy(
                            out=oc, in_=psv.rearrange("p a b -> p (a b)"))
                    else:
                        nc.scalar.copy(
                            out=oc, in_=psv.rearrange("p a b -> p (a b)"))
                    nc.sync.dma_start(out=outs_d[i][:, j * 512:(j + 1) * 512],
                                      in_=oc)
            return st

        # ---------------- per-branch finish chain ----------------
        def finish_a(st):
            pattn, kssb = st["pattn"], st["kssb"]
            # kk row norms -> rk, transposed into a broadcast row
            kn = small.tile([128, 1], F32, name="kn")
            nc.vector.tensor_reduce(out=kn, in_=kssb, axis=AX.X, op=ALU.add)
            nc.scalar.sqrt(kn, kn)
            nc.vector.tensor_scalar_max(kn, kn, EPS_NORM)
            rk = small.tile([128, 1], F32, name="rk")
            nc.vector.reciprocal(rk, kn)
            nc.vector.tensor_copy(out=rkmat[:, 0:1], in_=rk)
            psrk = ps.tile([128, 128], F32, name="psrk", tag="ps")
            nc.tensor.transpose(psrk, rkmat, ident_f)
            rkrow = small.tile([1, 128], F32, name="rkrow")
            nc.vector.tensor_copy(out=rkrow, in_=psrk[0:1, :])
            rkfull = med.tile([128, 128], F32, name="rkfull", tag="rkfull")
            nc.gpsimd.partition_broadcast(rkfull, rkrow)

            # z1 = attn_raw * rk[e];  stats of attn_s = z1 * rqs[c]
            z1 = med.tile([128, 128], F32, name="z1", tag="z1")
            nc.vector.tensor_mul(z1, pattn, rkfull)
            rs = small.tile([128, 1], F32, name="rs")
            nc.vector.tensor_reduce(out=rs, in_=z1, axis=AX.X, op=ALU.add)
            rs_s = small.tile([128, 1], F32, name="rs_s")
            nc.vector.tensor_mul(rs_s, rs, rqs)
            sq2 = scratch.tile([128, 128], F32, name="sq2", tag="sq")
            rss = small.tile([128, 1], F32, name="rss")
            nc.scalar.activation(out=sq2, in_=z1, func=ACTF.Square, accum_out=rss)
            rss_s = small.tile([128, 1], F32, name="rss_s")
            nc.vector.tensor_mul(rss_s, rss, rqs2)
            st2 = small.tile([128, 2], F32, name="st2")
            nc.vector.tensor_copy(out=st2[:, 0:1], in_=rs_s)
            nc.vector.tensor_copy(out=st2[:, 1:2], in_=rss_s)
            psst = ps.tile([1, 2], F32, name="psst", tag="ps")
            nc.tensor.matmul(psst, ones, st2)
            mu = small.tile([1, 1], F32, name="mu")
            nc.scalar.mul(mu, psst[0:1, 0:1], 1.0 / (C * C))
            ms = small.tile([1, 1], F32, name="ms")
            nc.scalar.mul(ms, psst[0:1, 1:2], 1.0 / (C * C))
            mu2 = small.tile([1, 1], F32, name="mu2")
            nc.scalar.square(mu2, mu)
            var = small.tile([1, 1], F32, name="var")
            nc.vector.tensor_sub(var, ms, mu2)
            std = small.tile([1, 1], F32, name="std")
            nc.scalar.activation(out=std, in_=var, func=ACTF.Sqrt, bias=epsin)
            rstd1 = small.tile([1, 1], F32, name="rstd1")
            nc.vector.reciprocal(rstd1, std)
            rstdf = small.tile([128, 1], F32, name="rstdf")
            nc.gpsimd.partition_broadcast(rstdf, rstd1)
            # softmax over e of z1*scale_c (instance-norm mean shift cancels)
            scale_c = small.tile([128, 1], F32, name="scale_c")
            nc.vector.tensor_mul(scale_c, rqs, rstdf)
            rm = small.tile([128, 1], F32, name="rm")
            nc.vector.tensor_reduce(out=rm, in_=z1, axis=AX.X, op=ALU.max)
            be = small.tile([128, 1], F32, name="be")
            nc.vector.tensor_mul(be, scale_c, rm)
            be_n = small.tile([128, 1], F32, name="be_n")
            nc.scalar.mul(be_n, be, -1.0)
            expb = med.tile([128, 128], F32, name="expb", tag="expb")
            se = small.tile([128, 1], F32, name="se")
            nc.scalar.activation(out=expb, in_=z1, func=ACTF.Exp,
                                 scale=scale_c, bias=be_n, accum_out=se)
            rse = small.tile([128, 1], F32, name="rse")
            nc.vector.reciprocal(rse, se)
            p_sb = med.tile([128, 128], F32, name="p_sb", tag="p_sb")
            nc.scalar.mul(p_sb, expb, rse)
            st["p_sb"] = p_sb

        def finish_b(st):
            i, p_sb = st["i"], st["p_sb"]
            # P2T = p.T @ Wpo_i.T  -> [e, o] operand for the weight fold
            psp2 = ps.tile([128, 128], F32, name="psp2", tag="ps")
            nc.tensor.matmul(psp2, p_sb, wpo_sb[:, i, :])
            p2t = med.tile([128, 128], BF16, name="p2t", tag="p2t")
            nc.vector.tensor_copy(out=p2t, in_=psp2)
            st["p2t"] = p2t

        def finish_w(st, batch):
            # What_t[c,o] = sum_e weffv_t[c,e] * (Wpo @ p)[o,e]; 3 taps/batch
            p2t, wvT_sb = st["p2t"], st["wvT_sb"]
            if batch == 0:
                st["what"] = whatp.tile([128, 9, 128], BF16, name="what",
                                        tag="what")
            what = st["what"]
            psw = ps.tile([128, 3, 128], F32, name="psw", tag="ps")
            for tt in range(3):
                t = 3 * batch + tt
                nc.tensor.matmul(psw[:, tt, :], wvT_sb[:, t, :], p2t,
                                 skip_group_check=True)
            nc.vector.tensor_copy(out=what[:, 3 * batch:3 * batch + 3, :],
                                  in_=psw)

        def make_fin_hook(st):
            def hook(s):
                if s == 2:
                    finish_a(st)
                elif s == 3:
                    finish_b(st)
                elif s in (4, 5, 6):
                    finish_w(st, s - 4)
            return hook

        def q_slab_hook(s):
            if s < 15:
                q_chunk(2 * s + 2)
                q_chunk(2 * s + 3)
            else:
                q_finalize()

        # ---------------- main schedule ----------------
        q_chunk(0)
        q_chunk(1)
        sts = {i: {"i": i} for i in range(4)}
        sts[0].update(prefetch_k(0))
        nc.sync.dma_start(out=wpo_sb, in_=wpoT_d.rearrange("i c o -> c i o"))

        # pass sequence: k0 k1 v0 k2 v1 k3 v2 v3
        seq = [("k", 0), ("k", 1), ("v", 0), ("k", 2), ("v", 1), ("k", 3),
               ("v", 2), ("v", 3)]

        def make_pre_hook(n):
            if n + 1 >= len(seq):
                return None
            kind, i = seq[n + 1]

            def hook():
                sts[i].update(prefetch_k(i) if kind == "k" else prefetch_v(i))
            return hook

        fin_i = [None]
        for n, (kind, i) in enumerate(seq):
            fin = make_fin_hook(sts[fin_i[0]]) if fin_i[0] is not None else None
            if kind == "k":
                kpass(i, sts[i], slab_hook=(q_slab_hook if n == 0 else None),
                      stage_hook=fin, pre_hook=make_pre_hook(n))
                fin_i[0] = i
            else:
                vpass(i, sts[i], stage_hook=fin, pre_hook=make_pre_hook(n))
                fin_i[0] = None


def _build_nc(has_bias):
    kp = 5 if has_bias else 4
    nc = bacc.Bacc("TRN2", target_bir_lowering=False, debug=False, num_devices=8)
    textT_d = nc.dram_tensor("textT", [kp * 128, C], FP8, kind="ExternalInput")
    wqT_d = nc.dram_tensor("wqT", [kp * 128, HW], FP8, kind="ExternalInput")
    embs_d = [nc.dram_tensor(f"emb{i}", [C, HW], BF16, kind="ExternalInput")
              for i in range(4)]
    embs8_d = [nc.dram_tensor(f"emb8_{i}", [C, HW], FP8, kind="ExternalInput")
               for i in range(4)]
    weffk8_d = nc.dram_tensor("weffk8", [4, 9, C, C], FP8, kind="ExternalInput")
    weffvT_d = nc.dram_tensor("weffvT", [4, 9, C, C], BF16, kind="ExternalInput")
    wpoT_d = nc.dram_tensor("wpoT", [4, C, C], F32, kind="ExternalInput")
    outs_d = [nc.dram_tensor(f"out{i}", [C, HW], F32, kind="ExternalOutput")
              for i in range(4)]
    with tile.TileContext(nc) as tc:
        _body(nc, tc, kp, textT_d, wqT_d, embs_d, embs8_d, weffk8_d, weffvT_d,
              wpoT_d, outs_d)
    nc.compile()
    return nc


_NC = {}
_HAS_BIAS = False


def _get_nc():
    if _HAS_BIAS not in _NC:
        _NC[_HAS_BIAS] = _build_nc(_HAS_BIAS)
    return _NC[_HAS_BIAS]


def _prep_in_maps(emb1, emb2, emb3, emb4, text_emb, Wq, bq, Wmk, Wk, Wmv, Wv, Wpo):
    global _HAS_BIAS
    f32 = np.float32
    embs = [np.ascontiguousarray(np.asarray(e, f32).reshape(B, C, HW))
            for e in (emb1, emb2, emb3, emb4)]
    text_emb = np.asarray(text_emb, f32)
    Wq = np.asarray(Wq, f32)
    bq = np.asarray(bq, f32)
    Wmk = np.asarray(Wmk, f32)
    Wk = np.asarray(Wk, f32)
    Wmv = np.asarray(Wmv, f32)
    Wv = np.asarray(Wv, f32)
    Wpo = np.asarray(Wpo, f32)

    _HAS_BIAS = bool(np.any(bq != 0.0))
    kp = 5 if _HAS_BIAS else 4

    # q path runs fp8: scale weights by 32 so q lands in fp8 range; the
    # scale cancels in the q l2norm.
    wqT = np.zeros((kp * 128, HW), f32)
    wqT[:TS] = Wq.T * 32.0
    if _HAS_BIAS:
        wqT[TS] = bq * 32.0
    wqT = np.clip(wqT, -240, 240).astype(E4_NP)

    g2 = (np.arange(C) // 2) * 2

    def build_weff(Wm, Wg):
        out = np.empty((4, 9, C, C), f32)
        for i in range(4):
            A = Wg[i][:, 0].reshape(C, 9)
            Bt = Wg[i][:, 1].reshape(C, 9)
            M0 = Wm[i][g2, :]
            M1 = Wm[i][g2 + 1, :]
            out[i] = (np.einsum('ot,oc->tco', A, M0)
                      + np.einsum('ot,oc->tco', Bt, M1)).astype(f32)
        return out

    weffk = build_weff(Wmk, Wk)
    # v weights transposed per tap: [t, e, c] for the on-device weight fold
    weffvT = np.ascontiguousarray(
        np.transpose(build_weff(Wmv, Wv), (0, 1, 3, 2)).astype(BF16_NP))
    # fp8 k weights: DoubleRow tap order, scaled by a per-branch power of two
    # (the scale cancels in the downstream l2norm)
    # fixed 256 scale keeps conv outputs within fp8 range for the attn path
    weffk8 = weffk[:, DR_ORDER] * 256.0
    weffk8 = np.ascontiguousarray(np.clip(weffk8, -240, 240).astype(E4_NP))
    wpoT = np.ascontiguousarray(np.transpose(Wpo, (0, 2, 1)))

    in_maps = []
    for b in range(B):
        textT = np.zeros((kp * 128, C), f32)
        textT[:TS] = text_emb[b, 0].T
        if _HAS_BIAS:
            textT[TS] = 1.0
        textT = np.clip(textT, -240, 240).astype(E4_NP)
        m = {"textT": textT, "wqT": wqT, "weffk8": weffk8, "weffvT": weffvT,
             "wpoT": wpoT}
        for i in range(4):
            eb = embs[i][b]
            m[f"emb{i}"] = np.ascontiguousarray(eb.astype(BF16_NP))
            m[f"emb8_{i}"] = np.ascontiguousarray(eb.astype(E4_NP))
        in_maps.append(m)
    return in_maps


def _run(in_maps, trace=False):
    nc = _get_nc()
    return bass_utils.run_bass_kernel_spmd(nc, in_maps, core_ids=list(range(8)),
                                           trace=trace)


def kernel(emb1, emb2, emb3, emb4, text_emb, Wq, bq, Wmk, Wk, Wmv, Wv, Wpo):
    in_maps = _prep_in_maps(emb1, emb2, emb3, emb4, text_emb, Wq, bq,
                            Wmk, Wk, Wmv, Wv, Wpo)
    res = _run(in_maps, trace=False)
    outs = []
    for i in range(4):
        o = np.stack([res.results[b][f"out{i}"].reshape(C, H, W)
                      for b in range(B)])
        outs.append(np.ascontiguousarray(o.astype(np.float32)))
    return tuple(outs)
